# revision 15
# baseline (speedup 1.0000x reference)
"""DoubleStreamBlock (flux-style) kernel for 8 trn2 NeuronCores.

Sharding plan:
  - The concatenated (cond, obs) sequence (4096 rows x 1024) is split into
    32 chunks of 128 rows; core r owns 4 stream-pure chunks g(r, s), s=0..3.
  - modulation + LN1 run row-sharded; the modulated activations are
    transposed and AllGathered (fp16) so every core holds full xm^T.
  - qkv / rmsnorm / rope / attention are head-sharded (2 of 16 heads per
    core, full L=4096).  Queries are processed in 4 slabs of 1024 in a
    permuted order such that each slab contains exactly one 128-row chunk
    per core; after each slab a small AllToAll returns attention rows to
    their owner so proj + LN2 + MLP of slab s overlap the (ScalarE-bound)
    softmax exp of slab s+1.
  - proj and MLP run row-sharded with full fp16 weights per core; no
    AllReduce anywhere.
Matmuls are fp16 (bf16 softmax numerator) with fp32 accumulation; all
norm / softmax arithmetic is fp32.
"""

import sys

if "/opt/trn_rl_repo" not in sys.path:
    sys.path.insert(0, "/opt/trn_rl_repo")

import numpy as np

import concourse.bass as bass
import concourse.mybir as mybir
import concourse.tile as tile
from concourse import bacc
from concourse.masks import make_identity

F16 = mybir.dt.float16
I32 = mybir.dt.int32
BF16 = mybir.dt.bfloat16
F32 = mybir.dt.float32
AF = mybir.ActivationFunctionType
OP = mybir.AluOpType
AX = mybir.AxisListType

P = 128
D = 1024
H = 16
DH = 64
MH = 4096
L = 4096
NC = 8
NSLAB = 4
KO = D // P
NM = MH // P
EPS = 1e-6
G0 = float(2.0 * np.sqrt(2.0 / np.pi))
G1 = float(2.0 * np.sqrt(2.0 / np.pi) * 0.044715)


def g_chunk(r, s):
    return 2 * s + r if r < 2 else 8 + 6 * s + (r - 2)


PI = [g_chunk(r, s) for s in range(NSLAB) for r in range(NC)]


def build_program():
    nc = bacc.Bacc("TRN2", target_bir_lowering=False, debug=False, num_devices=NC)

    # ---------------- I/O ----------------
    xr = nc.dram_tensor("xr", [NSLAB, P, D], F16, kind="ExternalInput")
    sv = nc.dram_tensor("sv", [P, KO], F32, kind="ExternalInput")
    wmod = nc.dram_tensor("wmod", [KO, P, 6 * D], F16, kind="ExternalInput")
    bmod = nc.dram_tensor("bmod", [1, 6 * D], F32, kind="ExternalInput")
    wqk = nc.dram_tensor("wqk", [2, KO, P, 256], F16, kind="ExternalInput")
    wv = nc.dram_tensor("wv", [2, KO, P, P], F16, kind="ExternalInput")
    pe_a = nc.dram_tensor("pe_a", [P, L], F16, kind="ExternalInput")
    pe_b = nc.dram_tensor("pe_b", [P, L], F16, kind="ExternalInput")
    qsc = nc.dram_tensor("qsc", [P, 2], F32, kind="ExternalInput")
    ksc = nc.dram_tensor("ksc", [P, 2], F32, kind="ExternalInput")
    wproj = nc.dram_tensor("wproj", [KO, P, D], F16, kind="ExternalInput")
    wm1 = nc.dram_tensor("wm1", [KO, P, MH], F16, kind="ExternalInput")
    wm2 = nc.dram_tensor("wm2", [NM, P, D], F16, kind="ExternalInput")
    b1c = nc.dram_tensor("b1c", [P, NM], F32, kind="ExternalInput")
    pb2 = nc.dram_tensor("pb2", [1, 2 * D], F32, kind="ExternalInput")
    y = nc.dram_tensor("y", [NSLAB, P, D], F32, kind="ExternalOutput")

    # internal DRAM
    modd = nc.dram_tensor("modd", [1, 6 * D], F32)
    agx_in = nc.dram_tensor("agx_in", [KO, P, 512], F16)
    agx_out = nc.dram_tensor("agx_out", [NC, KO, P, 512], F16, addr_space="Shared")
    a2a_in = nc.dram_tensor("a2a_in", [NSLAB, NC, P, P], F16)
    a2a_out = nc.dram_tensor("a2a_out", [NSLAB, NC, P, P], F16)

    # PSUM: 8 banks
    psA = nc.alloc_psum_tensor("psA", [P, 2048], F32)   # banks 0-3
    psB = nc.alloc_psum_tensor("psB", [P, 1536], F32)   # banks 4-6
    psC = nc.alloc_psum_tensor("psC", [P, 512], F32)    # bank 7

    RG = [list(range(NC))]
    PO_OFF = [65 * t if t < 7 else 512 + 65 * (t - 7) if t < 14 else 1024 + 65 * (t - 14)
              for t in range(16)]

    with tile.TileContext(nc) as tc, \
         tc.tile_pool(name="consts", bufs=1) as cpool, \
         tc.tile_pool(name="big", bufs=1) as bpool, \
         tc.tile_pool(name="scr", bufs=1) as spool:

        # ----- small persistent consts -----
        ident = cpool.tile([P, P], F32, tag="ident")
        make_identity(nc, ident[:])
        qsc_sb = cpool.tile([P, 2], F32, tag="qsc")
        ksc_sb = cpool.tile([P, 2], F32, tag="ksc")
        b1_sb = cpool.tile([P, NM], F32, tag="b1")
        ln1a = cpool.tile([P, KO], F32, tag="ln1a")
        ln1b = cpool.tile([P, KO], F32, tag="ln1b")
        eps_t = cpool.tile([P, 1], F32, tag="epst")
        nc.vector.memset(eps_t[:], EPS)
        c_g1 = cpool.tile([P, D], F16, tag="cg1")
        c_pb = cpool.tile([P, D], F16, tag="cpb")
        c_s2 = cpool.tile([P, D], F16, tag="cs2")
        c_sh2 = cpool.tile([P, D], F16, tag="csh2")
        c_g2 = cpool.tile([P, D], F16, tag="cg2")
        c_b2 = cpool.tile([P, D], F16, tag="cb2")
        bd1 = cpool.tile([P, P], F16, tag="bd1")
        nc.vector.memset(bd1[:], 0.0)
        nc.vector.memset(bd1[0:64, 0:64], 1.0)
        nc.vector.memset(bd1[64:128, 64:128], 1.0)

        # ----- big persistent -----
        qr = bpool.tile([P, L], F16, tag="qr")
        kr = bpool.tile([P, L], F16, tag="kr")
        vp = bpool.tile([P, 32, 130], BF16, tag="vp")
        w1_sb = bpool.tile([P, KO, MH], F16, tag="w1")
        hT_sb = bpool.tile([P, KO, P], F16, tag="hT")

        # ============ A0: silu(vec) + modulation matvec ============
        sv_sb = cpool.tile([P, KO], F32, tag="sv")
        nc.sync.dma_start(out=sv_sb[:], in_=sv[:])
        sil = cpool.tile([P, KO], F32, tag="sil")
        nc.scalar.activation(sil[:], sv_sb[:], AF.Exp, scale=-1.0)
        nc.vector.tensor_scalar(out=sil[:], in0=sil[:], scalar1=1.0, scalar2=None, op0=OP.add)
        silr = cpool.tile([P, KO], F32, tag="silr")
        nc.vector.reciprocal(silr[:], sil[:])
        s_sb = cpool.tile([P, KO], F16, tag="ssb")
        nc.vector.tensor_tensor(out=s_sb[:], in0=sv_sb[:], in1=silr[:], op=OP.mult)

        for rnd in range(3):
            for ko in range(KO):
                wmt = spool.tile([P, 2048], F16, tag="a4x", bufs=3)
                nc.sync.dma_start(out=wmt[:], in_=wmod[ko, :, 2048 * rnd : 2048 * (rnd + 1)])
                for g in range(4):
                    nc.tensor.matmul(
                        psA[0:1, 512 * g : 512 * (g + 1)],
                        lhsT=s_sb[:, ko : ko + 1],
                        rhs=wmt[:, 512 * g : 512 * (g + 1)],
                        start=(ko == 0), stop=(ko == KO - 1),
                    )
            bmt = spool.tile([1, 2048], F32, tag="a8b")
            nc.sync.dma_start(out=bmt[:], in_=bmod[0:1, 2048 * rnd : 2048 * (rnd + 1)])
            nc.vector.tensor_tensor(out=bmt[:], in0=psA[0:1, 0:2048], in1=bmt[:], op=OP.add)
            nc.sync.dma_start(out=modd[0:1, 2048 * rnd : 2048 * (rnd + 1)], in_=bmt[:])

        # ============ A1: broadcast const tiles from mod vectors ============
        for dst, lo in [(c_g1, 2 * D), (c_sh2, 3 * D), (c_s2, 4 * D), (c_g2, 5 * D)]:
            nc.gpsimd.dma_start(out=dst[:], in_=modd[0, lo : lo + D][None, :].broadcast_to((P, D)))
        nc.vector.tensor_scalar(out=c_s2[:], in0=c_s2[:], scalar1=1.0, scalar2=None, op0=OP.add)
        nc.sync.dma_start(out=ln1b[:], in_=modd[0, 0:D].rearrange("(k p) -> p k", p=P))
        nc.sync.dma_start(out=ln1a[:], in_=modd[0, D : 2 * D].rearrange("(k p) -> p k", p=P))
        nc.vector.tensor_scalar(out=ln1a[:], in0=ln1a[:], scalar1=1.0, scalar2=None, op0=OP.add)

        # ============ A2: LN1 -> xm^T -> AllGather ============
        xmt_sb = bpool.tile([P, KO, 512], F16, tag="gxmt")  # reused as g_sb later
        for s in range(NSLAB):
            xch = spool.tile([P, D], F16, tag="xrch", bufs=2)
            nc.sync.dma_start(out=xch[:], in_=xr[s])
            t1 = spool.tile([P, D], F32, tag="a4x", bufs=3)
            sqv = spool.tile([P, D], F32, tag="a4b")
            red = spool.tile([P, 6], F32, tag="red", bufs=2)
            nc.vector.reduce_sum(out=red[:, 0:1], in_=xch[:], axis=AX.X)
            nc.vector.tensor_tensor(out=sqv[:], in0=xch[:], in1=xch[:], op=OP.mult)
            nc.vector.reduce_sum(out=red[:, 1:2], in_=sqv[:], axis=AX.X)
            nc.vector.tensor_scalar(out=red[:, 2:3], in0=red[:, 0:1], scalar1=1.0 / D, scalar2=None, op0=OP.mult)
            nc.vector.tensor_scalar(out=red[:, 3:4], in0=red[:, 1:2], scalar1=1.0 / D, scalar2=None, op0=OP.mult)
            nc.vector.tensor_tensor(out=red[:, 4:5], in0=red[:, 2:3], in1=red[:, 2:3], op=OP.mult)
            nc.vector.tensor_tensor(out=red[:, 3:4], in0=red[:, 3:4], in1=red[:, 4:5], op=OP.subtract)
            nc.scalar.activation(red[:, 4:5], red[:, 3:4], AF.Ln, bias=eps_t[:])
            nc.scalar.activation(red[:, 5:6], red[:, 4:5], AF.Exp, scale=-0.5)
            nc.vector.tensor_scalar(
                out=t1[:], in0=xch[:], scalar1=red[:, 2:3], scalar2=red[:, 5:6],
                op0=OP.subtract, op1=OP.mult,
            )
            for g in range(2):
                for i in range(4):
                    ii = 4 * g + i
                    nc.tensor.matmul(
                        psA[:, 512 * g + 128 * i : 512 * g + 128 * (i + 1)],
                        lhsT=t1[:, 128 * ii : 128 * (ii + 1)], rhs=ident[:],
                        is_transpose=True, start=(i == 0), stop=(i == 3),
                    )
                for i in range(4):
                    ii = 4 * g + i
                    nc.vector.tensor_scalar(
                        out=xmt_sb[:, ii, 128 * s : 128 * (s + 1)],
                        in0=psA[:, 512 * g + 128 * i : 512 * g + 128 * (i + 1)],
                        scalar1=ln1a[:, ii : ii + 1], scalar2=ln1b[:, ii : ii + 1],
                        op0=OP.mult, op1=OP.add,
                    )
        nc.sync.dma_start(out=agx_in[:].rearrange("k p f -> p k f"), in_=xmt_sb[:])
        nc.gpsimd.collective_compute(
            "AllGather", OP.bypass, replica_groups=RG,
            ins=[agx_in[:].opt()], outs=[agx_out[:].opt()],
        )

        # qkv weights + pe planes (issued early so DMA overlaps phase A tail)
        pe_a_sb = spool.tile([P, L], F16, tag="a8", bufs=2)
        pe_b_sb = spool.tile([P, L], F16, tag="a8", bufs=2)
        wqkq = spool.tile([P, 2, KO, P], F16, tag="wqkq")
        wqkk = spool.tile([P, 2, KO, P], F16, tag="wqkk")
        wv_sb = spool.tile([P, 2, KO, P], F16, tag="wvsb")
        nc.sync.dma_start(out=wqkq[:], in_=wqk[:, :, :, 0:128].rearrange("t k p f -> p t k f"))
        nc.sync.dma_start(out=wqkk[:], in_=wqk[:, :, :, 128:256].rearrange("t k p f -> p t k f"))
        nc.sync.dma_start(out=wv_sb[:], in_=wv[:].rearrange("t k p f -> p t k f"))
        nc.sync.dma_start(out=pe_a_sb[:], in_=pe_a[:])
        nc.sync.dma_start(out=pe_b_sb[:], in_=pe_b[:])
        nc.sync.dma_start(out=qsc_sb[:], in_=qsc[:])
        nc.sync.dma_start(out=ksc_sb[:], in_=ksc[:])
        nc.sync.dma_start(out=b1_sb[:], in_=b1c[:])
        nc.gpsimd.dma_start(out=c_pb[:], in_=pb2[0, 0:D][None, :].broadcast_to((P, D)))
        nc.gpsimd.dma_start(out=c_b2[:], in_=pb2[0, D : 2 * D][None, :].broadcast_to((P, D)))
        for ko in range(KO):
            nc.sync.dma_start(out=w1_sb[:, ko, :], in_=wm1[ko])

        # ============ B: qkv (head-sharded) ============
        CGRP = [(0, 0), (256, 1), (512, 1), (768, 1)]

        nc.vector.memset(vp[:, :, 64:65], 1.0)
        nc.vector.memset(vp[:, :, 129:130], 1.0)
        for hq in range(4):          # one pi slab (1024 cols) per pass
            for ko in range(KO):
                xt = spool.tile([P, 1024], F16, tag="a4x", bufs=3)
                nc.sync.dma_start(
                    out=xt[:].rearrange("p (r i) -> p r i", r=NC),
                    in_=agx_out[:, ko, :, 128 * hq : 128 * (hq + 1)].rearrange("r p i -> p r i"),
                )
                for t, wt in [(0, wqkq), (1, wqkk), (2, wv_sb)]:
                    for (off, st) in CGRP:
                        if t == 0:
                            dst = psA[:, off : off + 256]
                        elif t == 1:
                            dst = psA[:, 1024 + off : 1024 + off + 256]
                        else:
                            dst = psB[:, off : off + 256]
                        nc.tensor.matmul(
                            dst,
                            lhsT=wt[:, st, ko, :],
                            rhs=xt[:, off : off + 256],
                            start=(ko == 0 and off % 512 == 0),
                            stop=(ko == KO - 1 and (off + 256) % 512 == 0),
                        )
            # ---- q then k: evict, rmsnorm, rope ----
            for t, (dst_r, scv) in enumerate([(qr, qsc_sb), (kr, ksc_sb)]):
                ev = spool.tile([P, 1024], F32, tag="a8b")
                nc.vector.tensor_copy(out=ev[:], in_=psA[:, 1024 * t : 1024 * (t + 1)])
                sq16 = spool.tile([P, 1024], F16, tag="a4c")
                nc.vector.tensor_tensor(out=sq16[:], in0=ev[:], in1=ev[:], op=OP.mult)
                nc.tensor.matmul(psB[:, 1024:1536], lhsT=bd1[:], rhs=sq16[:, 0:512], start=True, stop=True)
                nc.tensor.matmul(psC[:, 0:512], lhsT=bd1[:], rhs=sq16[:, 512:1024], start=True, stop=True)
                rst = spool.tile([P, 1024], F16, tag="a4b")
                nc.scalar.activation(rst[:, 0:512], psB[:, 1024:1536], AF.Ln, scale=1.0 / DH, bias=eps_t[:])
                nc.scalar.activation(rst[:, 512:1024], psC[:, 0:512], AF.Ln, scale=1.0 / DH, bias=eps_t[:])
                nc.scalar.activation(rst[:], rst[:], AF.Exp, scale=-0.5)
                qpre = spool.tile([P, 1024], F16, tag="a4c")
                nc.vector.tensor_tensor(out=qpre[:], in0=ev[:], in1=rst[:], op=OP.mult)
                nc.vector.tensor_scalar(out=qpre[:, 0:256], in0=qpre[:, 0:256],
                                        scalar1=scv[:, 0:1], scalar2=None, op0=OP.mult)
                nc.vector.tensor_scalar(out=qpre[:, 256:1024], in0=qpre[:, 256:1024],
                                        scalar1=scv[:, 1:2], scalar2=None, op0=OP.mult)
                dup = spool.tile([P, 1024], F16, tag="dup")
                tmp = spool.tile([P, 1024], F16, tag="a4b")
                for bl, sp in [(0, 0), (1, 0), (2, 64), (3, 64)]:
                    nc.sync.dma_start(out=dup[32 * bl : 32 * (bl + 1), :], in_=qpre[sp : sp + 32, :])
                nc.vector.tensor_tensor(out=tmp[:], in0=dup[:], in1=pe_a_sb[:, 1024 * hq : 1024 * (hq + 1)], op=OP.mult)
                for bl, sp in [(0, 32), (1, 32), (2, 96), (3, 96)]:
                    nc.sync.dma_start(out=dup[32 * bl : 32 * (bl + 1), :], in_=qpre[sp : sp + 32, :])
                nc.vector.tensor_tensor(out=dup[:], in0=dup[:], in1=pe_b_sb[:, 1024 * hq : 1024 * (hq + 1)], op=OP.mult)
                nc.vector.tensor_tensor(out=dst_r[:, 1024 * hq : 1024 * (hq + 1)], in0=tmp[:], in1=dup[:], op=OP.add)
            # ---- v: evict, transpose to [L, d], interleave ones ----
            vt32 = spool.tile([P, 1024], F32, tag="a8b")
            nc.vector.tensor_copy(out=vt32[:], in_=psB[:, 0:1024])
            for g in range(2):
                for i in range(4):
                    nc.tensor.matmul(
                        psB[:, 512 * g + 128 * i : 512 * g + 128 * (i + 1)],
                        lhsT=vt32[:, 128 * (4 * g + i) : 128 * (4 * g + i + 1)], rhs=ident[:],
                        is_transpose=True, start=(i == 0), stop=(i == 3),
                    )
                ch0 = 8 * hq + 4 * g
                dst = vp[:, ch0 : ch0 + 4, :].rearrange("p c (j x) -> p c j x", j=2, x=65)[:, :, :, 0:64]
                srcv = psB[:, 512 * g : 512 * (g + 1)].rearrange("p (c j x) -> p c j x", c=4, j=2, x=64)
                nc.vector.tensor_copy(out=dst, in_=srcv)

        # ============ C: attention + tail, per slab ============
        g_sb = xmt_sb  # same slot, reused as the gelu output buffer

        for s in range(NSLAB):
            for kc in range(32):
                for qh in range(2):
                    buf = (2 * kc + qh) % 2
                    base = 1024 * buf
                    qlo = 1024 * s + 512 * qh
                    nc.tensor.matmul(
                        psA[:, base : base + 512],
                        lhsT=kr[0:64, 128 * kc : 128 * (kc + 1)],
                        rhs=qr[0:64, qlo : qlo + 512],
                        start=True, stop=True, tile_position=(0, 0),
                    )
                    nc.tensor.matmul(
                        psA[:, base + 512 : base + 1024],
                        lhsT=kr[64:128, 128 * kc : 128 * (kc + 1)],
                        rhs=qr[64:128, qlo : qlo + 512],
                        start=True, stop=True, tile_position=(64, 0),
                    )
                    e_sb = spool.tile([P, 1024], BF16, tag="a4x", bufs=3)
                    nc.scalar.activation(e_sb[:], psA[:, base : base + 1024], AF.Exp, scale=0.125)
                    for h in range(2):
                        for bq in range(4):
                            b = 4 * qh + bq
                            tix = 8 * h + b
                            off = PO_OFF[tix]
                            nc.tensor.matmul(
                                psB[:, off : off + 65],
                                lhsT=e_sb[:, 512 * h + 128 * bq : 512 * h + 128 * (bq + 1)],
                                rhs=vp[:, kc, 65 * h : 65 * (h + 1)],
                                start=(kc == 0 and ((qh == 0 and tix in (0, 8)) or (qh == 1 and tix == 14))),
                                stop=(kc == 31 and qh == 1 and tix in (6, 13, 15)),
                            )
            # ---- evict + normalize + transpose + A2A ----
            po_sb = spool.tile([P, 1536], F32, tag="a8b")
            nc.vector.tensor_copy(out=po_sb[:], in_=psB[:])
            den = spool.tile([P, 16], F32, tag="den")
            nc.vector.tensor_copy(out=den[:, 0:7], in_=po_sb[:, 64:455:65])
            nc.vector.tensor_copy(out=den[:, 7:14], in_=po_sb[:, 576:967:65])
            nc.vector.tensor_copy(out=den[:, 14:16], in_=po_sb[:, 1088:1154:65])
            rec = spool.tile([P, 16], F32, tag="rec")
            nc.vector.reciprocal(rec[:], den[:])
            att = spool.tile([P, 8, P], F32, tag="a4b")
            for h in range(2):
                for b in range(8):
                    tix = 8 * h + b
                    nc.vector.tensor_scalar(
                        out=att[:, b, 64 * h : 64 * (h + 1)],
                        in0=po_sb[:, PO_OFF[tix] : PO_OFF[tix] + 64],
                        scalar1=rec[:, tix : tix + 1], scalar2=None, op0=OP.mult,
                    )
            for g in range(2):
                stage = spool.tile([P, 4, P], F16, tag="stage", bufs=2)
                for i in range(4):
                    nc.tensor.matmul(
                        psC[:, 128 * i : 128 * (i + 1)],
                        lhsT=att[:, 4 * g + i, :], rhs=ident[:],
                        is_transpose=True, start=(i == 0), stop=(i == 3),
                    )
                nc.vector.tensor_copy(out=stage[:].rearrange("p c f -> p (c f)"), in_=psC[:, 0:512])
                nc.sync.dma_start(
                    out=a2a_in[s, 4 * g : 4 * (g + 1)].rearrange("c p f -> p c f"),
                    in_=stage[:],
                )
            nc.gpsimd.collective_compute(
                "AllToAll", OP.bypass, replica_groups=RG,
                ins=[a2a_in[s].opt()], outs=[a2a_out[s].opt()],
            )

            # ---- proj + residual ----
            at_sb = spool.tile([P, KO, P], F16, tag="atsb", bufs=2)
            nc.sync.dma_start(out=at_sb[:], in_=a2a_out[s].rearrange("r p f -> p r f"))
            x2 = spool.tile([P, D], F32, tag="wqkq")
            xc2 = spool.tile([P, D], F16, tag="xrch", bufs=2)
            nc.sync.dma_start(out=xc2[:], in_=xr[s])
            for nh in range(2):
                wpq = spool.tile([P, KO, 512], F16, tag="a8", bufs=2)
                nc.sync.dma_start(
                    out=wpq[:],
                    in_=wproj[:, :, 512 * nh : 512 * (nh + 1)].rearrange("k p f -> p k f"),
                )
                for ko in range(KO):
                    nc.tensor.matmul(
                        psC[:, 0:512],
                        lhsT=at_sb[:, ko, :],
                        rhs=wpq[:, ko, :],
                        start=(ko == 0), stop=(ko == KO - 1),
                    )
                sl = slice(512 * nh, 512 * (nh + 1))
                nc.vector.tensor_tensor(out=x2[:, sl], in0=psC[:, 0:512], in1=c_pb[:, sl], op=OP.add)
                nc.vector.tensor_tensor(out=x2[:, sl], in0=x2[:, sl], in1=c_g1[:, sl], op=OP.mult)
                nc.vector.tensor_tensor(out=x2[:, sl], in0=x2[:, sl], in1=xc2[:, sl], op=OP.add)

            # ---- LN2 + modulation -> h^T ----
            h32 = spool.tile([P, D], F32, tag="wqkk")
            red2 = spool.tile([P, 6], F32, tag="red", bufs=2)
            nc.vector.reduce_sum(out=red2[:, 0:1], in_=x2[:], axis=AX.X)
            nc.vector.tensor_tensor(out=h32[:], in0=x2[:], in1=x2[:], op=OP.mult)
            nc.vector.reduce_sum(out=red2[:, 1:2], in_=h32[:], axis=AX.X)
            nc.vector.tensor_scalar(out=red2[:, 2:3], in0=red2[:, 0:1], scalar1=1.0 / D, scalar2=None, op0=OP.mult)
            nc.vector.tensor_scalar(out=red2[:, 3:4], in0=red2[:, 1:2], scalar1=1.0 / D, scalar2=None, op0=OP.mult)
            nc.vector.tensor_tensor(out=red2[:, 4:5], in0=red2[:, 2:3], in1=red2[:, 2:3], op=OP.mult)
            nc.vector.tensor_tensor(out=red2[:, 3:4], in0=red2[:, 3:4], in1=red2[:, 4:5], op=OP.subtract)
            # rstd = rsqrt(var + eps) on DVE (magic-constant + 3 Newton steps)
            nc.vector.tensor_scalar(out=red2[:, 4:5], in0=red2[:, 3:4], scalar1=EPS, scalar2=None, op0=OP.add)
            ri = spool.tile([P, 2], I32, tag="ri")
            nc.vector.tensor_scalar(out=ri[:, 0:1], in0=red2[:, 4:5].bitcast(I32), scalar1=1, scalar2=None, op0=OP.logical_shift_right)
            nc.vector.tensor_scalar(out=ri[:, 0:1], in0=ri[:, 0:1], scalar1=-1, scalar2=0x5F3759DF, op0=OP.mult, op1=OP.add)
            rf = ri[:, 0:1].bitcast(F32)
            nt = spool.tile([P, 1], F32, tag="nt")
            for _ in range(3):
                nc.vector.tensor_tensor(out=nt[:], in0=red2[:, 4:5], in1=rf, op=OP.mult)
                nc.vector.tensor_tensor(out=nt[:], in0=nt[:], in1=rf, op=OP.mult)
                nc.vector.tensor_scalar(out=nt[:], in0=nt[:], scalar1=-0.5, scalar2=1.5, op0=OP.mult, op1=OP.add)
                nc.vector.tensor_tensor(out=ri[:, 0:1].bitcast(F32), in0=rf, in1=nt[:], op=OP.mult)
            nc.vector.tensor_copy(out=red2[:, 5:6], in_=rf)
            nc.vector.tensor_scalar(
                out=h32[:], in0=x2[:], scalar1=red2[:, 2:3], scalar2=red2[:, 5:6],
                op0=OP.subtract, op1=OP.mult,
            )
            nc.vector.tensor_tensor(out=h32[:], in0=h32[:], in1=c_s2[:], op=OP.mult)
            nc.vector.tensor_tensor(out=h32[:], in0=h32[:], in1=c_sh2[:], op=OP.add)
            for g in range(2):
                for i in range(4):
                    ii = 4 * g + i
                    nc.tensor.matmul(
                        psC[:, 128 * i : 128 * (i + 1)],
                        lhsT=h32[:, 128 * ii : 128 * (ii + 1)], rhs=ident[:],
                        is_transpose=True, start=(i == 0), stop=(i == 3),
                    )
                nc.vector.tensor_copy(
                    out=hT_sb[:, 4 * g : 4 * (g + 1), :].rearrange("p c f -> p (c f)"),
                    in_=psC[:, 0:512],
                )

            # ---- MLP up (w1) + gelu ----
            gflat = g_sb[:].rearrange("p k f -> p (k f)")
            for mq in range(8):
                for mi in range(4):
                    m = 4 * mq + mi
                    for ko in range(KO):
                        nc.tensor.matmul(
                            psC[:, 128 * mi : 128 * (mi + 1)],
                            lhsT=w1_sb[:, ko, 128 * m : 128 * (m + 1)],
                            rhs=hT_sb[:, ko, :],
                            start=(mi == 0 and ko == 0),
                            stop=(mi == 3 and ko == KO - 1),
                        )
                ptmp = spool.tile([P, 512], F32, tag="ptmp", bufs=2)
                nc.vector.tensor_copy(out=ptmp[:], in_=psC[:, 0:512])
                xg4 = spool.tile([P, 512], F16, tag="xg4")
                for mi in range(4):
                    m = 4 * mq + mi
                    nc.vector.tensor_scalar(
                        out=xg4[:, 128 * mi : 128 * (mi + 1)],
                        in0=ptmp[:, 128 * mi : 128 * (mi + 1)],
                        scalar1=b1_sb[:, m : m + 1], scalar2=None, op0=OP.add,
                    )
                u4 = spool.tile([P, 512], F16, tag="u4")
                nc.vector.tensor_tensor(out=u4[:], in0=xg4[:], in1=xg4[:], op=OP.mult)
                nc.vector.tensor_scalar(out=u4[:], in0=u4[:], scalar1=G1, scalar2=G0, op0=OP.mult, op1=OP.add)
                nc.vector.tensor_tensor(out=u4[:], in0=xg4[:], in1=u4[:], op=OP.mult)
                d4 = spool.tile([P, 512], F32, tag="d4", bufs=2)
                nc.scalar.activation(d4[:], u4[:], AF.Exp, scale=-1.0)
                nc.vector.tensor_scalar(out=d4[:], in0=d4[:], scalar1=1.0, scalar2=None, op0=OP.add)
                r4 = spool.tile([P, 512], F32, tag="r4")
                nc.vector.reciprocal(r4[:], d4[:])
                nc.vector.tensor_tensor(
                    out=gflat[:, 512 * mq : 512 * (mq + 1)],
                    in0=xg4[:], in1=r4[:], op=OP.mult,
                )

            # ---- MLP down (w2) + residual -> y ----
            out_sb = spool.tile([P, D], F32, tag="outsb")
            for nh in range(2):
                for mq4 in range(4):
                    w2q = spool.tile([P, 8, 512], F16, tag="a8", bufs=2)
                    nc.sync.dma_start(
                        out=w2q[:],
                        in_=wm2[8 * mq4 : 8 * (mq4 + 1), :, 512 * nh : 512 * (nh + 1)].rearrange("m p f -> p m f"),
                    )
                    for mm in range(8):
                        m = 8 * mq4 + mm
                        nc.tensor.matmul(
                            psC[:, 0:512],
                            lhsT=gflat[:, 128 * m : 128 * (m + 1)],
                            rhs=w2q[:, mm, :],
                            start=(m == 0), stop=(m == 31),
                        )
                sl = slice(512 * nh, 512 * (nh + 1))
                nc.vector.tensor_tensor(out=out_sb[:, sl], in0=psC[:, 0:512], in1=c_b2[:, sl], op=OP.add)
                nc.vector.tensor_tensor(out=out_sb[:, sl], in0=out_sb[:, sl], in1=c_g2[:, sl], op=OP.mult)
                nc.vector.tensor_tensor(out=out_sb[:, sl], in0=out_sb[:, sl], in1=x2[:, sl], op=OP.add)
            nc.sync.dma_start(out=y[s], in_=out_sb[:])

    nc.compile()
    return nc


# ======================= host side =======================

_PROG = None


def _get_program():
    global _PROG
    if _PROG is None:
        _PROG = build_program()
    return _PROG


def _qk_cols(h0):
    idx = []
    for hh in (h0, h0 + 1):
        idx += [hh * 64 + 2 * p for p in range(32)]
        idx += [hh * 64 + 2 * p + 1 for p in range(32)]
    return idx


def _prep_core(c, x_full, vec, pe0, w):
    f16 = np.float16
    d = {}
    chunks = [g_chunk(c, s) for s in range(NSLAB)]
    d["xr"] = np.stack([x_full[128 * g : 128 * (g + 1)] for g in chunks]).astype(f16)
    d["sv"] = vec.reshape(KO, P).T.astype(np.float32).copy()
    st = "cond" if c < 2 else "obs"
    d["wmod"] = w[f"{st}_mod_w"].reshape(KO, P, 6 * D).astype(f16)
    d["bmod"] = w[f"{st}_mod_b"].reshape(1, 6 * D).astype(np.float32)
    qc = _qk_cols(2 * c)
    kc = [D + j for j in qc]
    vc = [2 * D + 64 * (2 * c) + i for i in range(128)]
    wqk = np.stack(
        [
            np.concatenate([w["cond_qkv_w"][:, qc], w["cond_qkv_w"][:, kc]], axis=1),
            np.concatenate([w["obs_qkv_w"][:, qc], w["obs_qkv_w"][:, kc]], axis=1),
        ]
    )
    d["wqk"] = wqk.reshape(2, KO, P, 256).astype(f16)
    wvv = np.stack([w["cond_qkv_w"][:, vc], w["obs_qkv_w"][:, vc]])
    d["wv"] = wvv.reshape(2, KO, P, P).astype(f16)
    perm = np.concatenate([np.arange(128 * g, 128 * (g + 1)) for g in PI])
    peP = pe0[perm]
    pair = np.arange(P) % 32
    jout = (np.arange(P) // 32) % 2
    d["pe_a"] = peP[:, pair, jout, 0].T.astype(f16).copy()
    d["pe_b"] = peP[:, pair, jout, 1].T.astype(f16).copy()
    dmap = 2 * (np.arange(P) % 32) + ((np.arange(P) // 32) % 2)
    d["qsc"] = np.stack([w["cond_q_scale"][dmap], w["obs_q_scale"][dmap]], axis=1).astype(np.float32).copy()
    d["ksc"] = np.stack([w["cond_k_scale"][dmap], w["obs_k_scale"][dmap]], axis=1).astype(np.float32).copy()
    d["wproj"] = w[f"{st}_proj_w"].reshape(KO, P, D).astype(f16)
    d["wm1"] = w[f"{st}_mlp_w1"].reshape(KO, P, MH).astype(f16)
    d["wm2"] = w[f"{st}_mlp_w2"].reshape(NM, P, D).astype(f16)
    d["b1c"] = w[f"{st}_mlp_b1"].reshape(NM, P).T.astype(np.float32).copy()
    d["pb2"] = np.concatenate([w[f"{st}_proj_b"], w[f"{st}_mlp_b2"]]).reshape(1, 2 * D).astype(np.float32)
    return d


def kernel(**inputs):
    nc = _get_program()
    from concourse.bass_utils import run_bass_kernel_spmd

    w = {k: np.asarray(v) for k, v in inputs.items()}
    obs = w["obs"][0].astype(np.float32)
    cond = w["cond"][0].astype(np.float32)
    x_full = np.concatenate([cond, obs], axis=0)
    vec = w["vec"][0].astype(np.float32)
    pe0 = w["pe"][0, 0].astype(np.float32)

    in_maps = [_prep_core(c, x_full, vec, pe0, w) for c in range(NC)]
    res = run_bass_kernel_spmd(nc, in_maps, list(range(NC)), trace=False)

    out_full = np.zeros((L, D), np.float32)
    for r in range(NC):
        yr = res.results[r]["y"]
        for s in range(NSLAB):
            g = g_chunk(r, s)
            out_full[128 * g : 128 * (g + 1)] = yr[s]
    return out_full[1024:][None], out_full[:1024][None]


# revision 17
# speedup vs baseline: 1.0792x; 1.0792x over previous
"""DoubleStreamBlock (flux-style) kernel for 8 trn2 NeuronCores.

Sharding plan:
  - The concatenated (cond, obs) sequence (4096 rows x 1024) is split into
    32 chunks of 128 rows; core r owns 4 stream-pure chunks g(r, s), s=0..3.
  - modulation + LN1 run row-sharded; the modulated activations are
    transposed and AllGathered (fp16) so every core holds full xm^T.
  - qkv / rmsnorm / rope / attention are head-sharded (2 of 16 heads per
    core, full L=4096).  Queries are processed in 4 slabs of 1024 in a
    permuted order such that each slab contains exactly one 128-row chunk
    per core; after each slab a small AllToAll returns attention rows to
    their owner so proj + LN2 + MLP of slab s overlap the (ScalarE-bound)
    softmax exp of slab s+1.
  - proj and MLP run row-sharded with full fp16 weights per core; no
    AllReduce anywhere.
Matmuls are fp16 (bf16 softmax numerator) with fp32 accumulation; all
norm / softmax arithmetic is fp32.
"""

import sys

if "/opt/trn_rl_repo" not in sys.path:
    sys.path.insert(0, "/opt/trn_rl_repo")

import numpy as np

import concourse.bass as bass
import concourse.mybir as mybir
import concourse.tile as tile
from concourse import bacc
from concourse.masks import make_identity

F16 = mybir.dt.float16
I32 = mybir.dt.int32
BF16 = mybir.dt.bfloat16
F32 = mybir.dt.float32
AF = mybir.ActivationFunctionType
OP = mybir.AluOpType
AX = mybir.AxisListType

P = 128
D = 1024
H = 16
DH = 64
MH = 4096
L = 4096
NC = 8
NSLAB = 4
KO = D // P
NM = MH // P
EPS = 1e-6
G0 = float(2.0 * np.sqrt(2.0 / np.pi))
G1 = float(2.0 * np.sqrt(2.0 / np.pi) * 0.044715)


def g_chunk(r, s):
    return 2 * s + r if r < 2 else 8 + 6 * s + (r - 2)


PI = [g_chunk(r, s) for s in range(NSLAB) for r in range(NC)]


def build_program():
    nc = bacc.Bacc("TRN2", target_bir_lowering=False, debug=False, num_devices=NC)

    # ---------------- I/O ----------------
    xr = nc.dram_tensor("xr", [NSLAB, P, D], F16, kind="ExternalInput")
    sv = nc.dram_tensor("sv", [P, KO], F32, kind="ExternalInput")
    wmod = nc.dram_tensor("wmod", [KO, P, 6 * D], F16, kind="ExternalInput")
    bmod = nc.dram_tensor("bmod", [1, 6 * D], F32, kind="ExternalInput")
    wqk = nc.dram_tensor("wqk", [2, KO, P, 256], F16, kind="ExternalInput")
    wv = nc.dram_tensor("wv", [2, KO, P, P], F16, kind="ExternalInput")
    pe_a = nc.dram_tensor("pe_a", [P, L], F16, kind="ExternalInput")
    pe_b = nc.dram_tensor("pe_b", [P, L], F16, kind="ExternalInput")
    qsc = nc.dram_tensor("qsc", [P, 2], F32, kind="ExternalInput")
    ksc = nc.dram_tensor("ksc", [P, 2], F32, kind="ExternalInput")
    wproj = nc.dram_tensor("wproj", [KO, P, D], F16, kind="ExternalInput")
    wm1 = nc.dram_tensor("wm1", [KO, P, MH], F16, kind="ExternalInput")
    wm2 = nc.dram_tensor("wm2", [NM, P, D], F16, kind="ExternalInput")
    b1c = nc.dram_tensor("b1c", [P, NM], F32, kind="ExternalInput")
    pb2 = nc.dram_tensor("pb2", [1, 2 * D], F32, kind="ExternalInput")
    y = nc.dram_tensor("y", [NSLAB, P, D], F32, kind="ExternalOutput")

    # internal DRAM
    modd = nc.dram_tensor("modd", [1, 6 * D], F32)
    agx_in = nc.dram_tensor("agx_in", [KO, P, 512], F16)
    agx_out = nc.dram_tensor("agx_out", [NC, KO, P, 512], F16, addr_space="Shared")
    a2a_in = nc.dram_tensor("a2a_in", [NSLAB, NC, P, P], F16)
    a2a_out = nc.dram_tensor("a2a_out", [NSLAB, NC, P, P], F16)

    # PSUM: 8 banks
    psA = nc.alloc_psum_tensor("psA", [P, 2048], F32)   # banks 0-3
    psB = nc.alloc_psum_tensor("psB", [P, 1536], F32)   # banks 4-6
    psC = nc.alloc_psum_tensor("psC", [P, 512], F32)    # bank 7

    RG = [list(range(NC))]
    PO_OFF = [65 * t if t < 7 else 512 + 65 * (t - 7) if t < 14 else 1024 + 65 * (t - 14)
              for t in range(16)]

    with tile.TileContext(nc) as tc, \
         tc.tile_pool(name="consts", bufs=1) as cpool, \
         tc.tile_pool(name="big", bufs=1) as bpool, \
         tc.tile_pool(name="scr", bufs=1) as spool:

        # ----- small persistent consts -----
        ident = cpool.tile([P, P], F32, tag="ident")
        make_identity(nc, ident[:])
        qsc_sb = cpool.tile([P, 2], F32, tag="qsc")
        ksc_sb = cpool.tile([P, 2], F32, tag="ksc")
        b1_sb = cpool.tile([P, NM], F32, tag="b1")
        ln1a = cpool.tile([P, KO], F32, tag="ln1a")
        ln1b = cpool.tile([P, KO], F32, tag="ln1b")
        eps_t = cpool.tile([P, 1], F32, tag="epst")
        nc.vector.memset(eps_t[:], EPS)
        c_g1 = cpool.tile([P, D], F16, tag="cg1")
        c_pb = cpool.tile([P, D], F16, tag="cpb")
        c_s2 = cpool.tile([P, D], F16, tag="cs2")
        c_sh2 = cpool.tile([P, D], F16, tag="csh2")
        c_g2 = cpool.tile([P, D], F16, tag="cg2")
        c_b2 = cpool.tile([P, D], F16, tag="cb2")
        bd1 = cpool.tile([P, P], F16, tag="bd1")
        nc.vector.memset(bd1[:], 0.0)
        nc.vector.memset(bd1[0:64, 0:64], 1.0)
        nc.vector.memset(bd1[64:128, 64:128], 1.0)

        # ----- big persistent -----
        qr = bpool.tile([P, L], F16, tag="qr")
        kr = bpool.tile([P, L], F16, tag="kr")
        vp = bpool.tile([P, 32, 130], BF16, tag="vp")
        w1_sb = bpool.tile([P, KO, MH], F16, tag="w1")
        hT_sb = bpool.tile([P, KO, P], F16, tag="hT")

        # ============ A0: silu(vec) + modulation matvec ============
        sv_sb = cpool.tile([P, KO], F32, tag="sv")
        nc.sync.dma_start(out=sv_sb[:], in_=sv[:])
        sil = cpool.tile([P, KO], F32, tag="sil")
        nc.scalar.activation(sil[:], sv_sb[:], AF.Exp, scale=-1.0)
        nc.vector.tensor_scalar(out=sil[:], in0=sil[:], scalar1=1.0, scalar2=None, op0=OP.add)
        silr = cpool.tile([P, KO], F32, tag="silr")
        nc.vector.reciprocal(silr[:], sil[:])
        s_sb = cpool.tile([P, KO], F16, tag="ssb")
        nc.vector.tensor_tensor(out=s_sb[:], in0=sv_sb[:], in1=silr[:], op=OP.mult)

        for rnd in range(3):
            for ko in range(KO):
                wmt = spool.tile([P, 2048], F16, tag="a4x", bufs=3)
                nc.sync.dma_start(out=wmt[:], in_=wmod[ko, :, 2048 * rnd : 2048 * (rnd + 1)])
                for g in range(4):
                    nc.tensor.matmul(
                        psA[0:1, 512 * g : 512 * (g + 1)],
                        lhsT=s_sb[:, ko : ko + 1],
                        rhs=wmt[:, 512 * g : 512 * (g + 1)],
                        start=(ko == 0), stop=(ko == KO - 1),
                    )
            bmt = spool.tile([1, 2048], F32, tag="a8b")
            nc.sync.dma_start(out=bmt[:], in_=bmod[0:1, 2048 * rnd : 2048 * (rnd + 1)])
            nc.vector.tensor_tensor(out=bmt[:], in0=psA[0:1, 0:2048], in1=bmt[:], op=OP.add)
            nc.sync.dma_start(out=modd[0:1, 2048 * rnd : 2048 * (rnd + 1)], in_=bmt[:])

        # ============ A1: broadcast const tiles from mod vectors ============
        for dst, lo in [(c_g1, 2 * D), (c_sh2, 3 * D), (c_s2, 4 * D), (c_g2, 5 * D)]:
            nc.gpsimd.dma_start(out=dst[:], in_=modd[0, lo : lo + D][None, :].broadcast_to((P, D)))
        nc.vector.tensor_scalar(out=c_s2[:], in0=c_s2[:], scalar1=1.0, scalar2=None, op0=OP.add)
        nc.sync.dma_start(out=ln1b[:], in_=modd[0, 0:D].rearrange("(k p) -> p k", p=P))
        nc.sync.dma_start(out=ln1a[:], in_=modd[0, D : 2 * D].rearrange("(k p) -> p k", p=P))
        nc.vector.tensor_scalar(out=ln1a[:], in0=ln1a[:], scalar1=1.0, scalar2=None, op0=OP.add)

        # ============ A2: LN1 -> xm^T -> AllGather ============
        xmt_sb = bpool.tile([P, KO, 512], F16, tag="gxmt")  # reused as g_sb later
        for s in range(NSLAB):
            xch = spool.tile([P, D], F16, tag="xrch", bufs=2)
            nc.sync.dma_start(out=xch[:], in_=xr[s])
            t1 = spool.tile([P, D], F32, tag="a4x", bufs=3)
            sqv = spool.tile([P, D], F32, tag="a4b")
            red = spool.tile([P, 6], F32, tag="red", bufs=2)
            nc.vector.reduce_sum(out=red[:, 0:1], in_=xch[:], axis=AX.X)
            nc.vector.tensor_tensor(out=sqv[:], in0=xch[:], in1=xch[:], op=OP.mult)
            nc.vector.reduce_sum(out=red[:, 1:2], in_=sqv[:], axis=AX.X)
            nc.vector.tensor_scalar(out=red[:, 2:3], in0=red[:, 0:1], scalar1=1.0 / D, scalar2=None, op0=OP.mult)
            nc.vector.tensor_scalar(out=red[:, 3:4], in0=red[:, 1:2], scalar1=1.0 / D, scalar2=None, op0=OP.mult)
            nc.vector.tensor_tensor(out=red[:, 4:5], in0=red[:, 2:3], in1=red[:, 2:3], op=OP.mult)
            nc.vector.tensor_tensor(out=red[:, 3:4], in0=red[:, 3:4], in1=red[:, 4:5], op=OP.subtract)
            nc.scalar.activation(red[:, 4:5], red[:, 3:4], AF.Ln, bias=eps_t[:])
            nc.scalar.activation(red[:, 5:6], red[:, 4:5], AF.Exp, scale=-0.5)
            nc.vector.tensor_scalar(
                out=t1[:], in0=xch[:], scalar1=red[:, 2:3], scalar2=red[:, 5:6],
                op0=OP.subtract, op1=OP.mult,
            )
            for g in range(2):
                for i in range(4):
                    ii = 4 * g + i
                    nc.tensor.matmul(
                        psA[:, 512 * g + 128 * i : 512 * g + 128 * (i + 1)],
                        lhsT=t1[:, 128 * ii : 128 * (ii + 1)], rhs=ident[:],
                        is_transpose=True, start=(i == 0), stop=(i == 3),
                    )
                for i in range(4):
                    ii = 4 * g + i
                    nc.vector.tensor_scalar(
                        out=xmt_sb[:, ii, 128 * s : 128 * (s + 1)],
                        in0=psA[:, 512 * g + 128 * i : 512 * g + 128 * (i + 1)],
                        scalar1=ln1a[:, ii : ii + 1], scalar2=ln1b[:, ii : ii + 1],
                        op0=OP.mult, op1=OP.add,
                    )
        nc.sync.dma_start(out=agx_in[:].rearrange("k p f -> p k f"), in_=xmt_sb[:])
        nc.gpsimd.collective_compute(
            "AllGather", OP.bypass, replica_groups=RG,
            ins=[agx_in[:].opt()], outs=[agx_out[:].opt()],
        )

        # qkv weights + pe planes (issued early so DMA overlaps phase A tail)
        pe_a_sb = spool.tile([P, L], F16, tag="a8", bufs=2)
        pe_b_sb = spool.tile([P, L], F16, tag="a8", bufs=2)
        wqkq = spool.tile([P, 2, KO, P], F16, tag="wqkq")
        wqkk = spool.tile([P, 2, KO, P], F16, tag="wqkk")
        wv_sb = spool.tile([P, 2, KO, P], F16, tag="wvsb")
        nc.sync.dma_start(out=wqkq[:], in_=wqk[:, :, :, 0:128].rearrange("t k p f -> p t k f"))
        nc.sync.dma_start(out=wqkk[:], in_=wqk[:, :, :, 128:256].rearrange("t k p f -> p t k f"))
        nc.sync.dma_start(out=wv_sb[:], in_=wv[:].rearrange("t k p f -> p t k f"))
        nc.sync.dma_start(out=pe_a_sb[:], in_=pe_a[:])
        nc.sync.dma_start(out=pe_b_sb[:], in_=pe_b[:])
        nc.sync.dma_start(out=qsc_sb[:], in_=qsc[:])
        nc.sync.dma_start(out=ksc_sb[:], in_=ksc[:])
        nc.sync.dma_start(out=b1_sb[:], in_=b1c[:])
        nc.gpsimd.dma_start(out=c_pb[:], in_=pb2[0, 0:D][None, :].broadcast_to((P, D)))
        nc.gpsimd.dma_start(out=c_b2[:], in_=pb2[0, D : 2 * D][None, :].broadcast_to((P, D)))
        for ko in range(KO):
            nc.sync.dma_start(out=w1_sb[:, ko, :], in_=wm1[ko])

        # ============ B: qkv (head-sharded) ============
        CGRP = [(0, 0), (256, 1), (512, 1), (768, 1)]

        nc.vector.memset(vp[:, :, 64:65], 1.0)
        nc.vector.memset(vp[:, :, 129:130], 1.0)
        for hq in range(4):          # one pi slab (1024 cols) per pass
            for ko in range(KO):
                xt = spool.tile([P, 1024], F16, tag="a4x", bufs=3)
                nc.sync.dma_start(
                    out=xt[:].rearrange("p (r i) -> p r i", r=NC),
                    in_=agx_out[:, ko, :, 128 * hq : 128 * (hq + 1)].rearrange("r p i -> p r i"),
                )
                for t, wt in [(0, wqkq), (1, wqkk), (2, wv_sb)]:
                    for (off, st) in CGRP:
                        if t == 0:
                            dst = psA[:, off : off + 256]
                        elif t == 1:
                            dst = psA[:, 1024 + off : 1024 + off + 256]
                        else:
                            dst = psB[:, off : off + 256]
                        nc.tensor.matmul(
                            dst,
                            lhsT=wt[:, st, ko, :],
                            rhs=xt[:, off : off + 256],
                            start=(ko == 0 and off % 512 == 0),
                            stop=(ko == KO - 1 and (off + 256) % 512 == 0),
                        )
            # ---- q then k: evict, rmsnorm, rope ----
            for t, (dst_r, scv) in enumerate([(qr, qsc_sb), (kr, ksc_sb)]):
                ev = spool.tile([P, 1024], F32, tag="a8b")
                nc.vector.tensor_copy(out=ev[:], in_=psA[:, 1024 * t : 1024 * (t + 1)])
                sq16 = spool.tile([P, 1024], F16, tag="a4c")
                nc.vector.tensor_tensor(out=sq16[:], in0=ev[:], in1=ev[:], op=OP.mult)
                nc.tensor.matmul(psB[:, 1024:1536], lhsT=bd1[:], rhs=sq16[:, 0:512], start=True, stop=True)
                nc.tensor.matmul(psC[:, 0:512], lhsT=bd1[:], rhs=sq16[:, 512:1024], start=True, stop=True)
                rst = spool.tile([P, 1024], F16, tag="a4b")
                nc.scalar.activation(rst[:, 0:512], psB[:, 1024:1536], AF.Ln, scale=1.0 / DH, bias=eps_t[:])
                nc.scalar.activation(rst[:, 512:1024], psC[:, 0:512], AF.Ln, scale=1.0 / DH, bias=eps_t[:])
                nc.scalar.activation(rst[:], rst[:], AF.Exp, scale=-0.5)
                qpre = spool.tile([P, 1024], F16, tag="a4c")
                nc.vector.tensor_tensor(out=qpre[:], in0=ev[:], in1=rst[:], op=OP.mult)
                nc.vector.tensor_scalar(out=qpre[:, 0:256], in0=qpre[:, 0:256],
                                        scalar1=scv[:, 0:1], scalar2=None, op0=OP.mult)
                nc.vector.tensor_scalar(out=qpre[:, 256:1024], in0=qpre[:, 256:1024],
                                        scalar1=scv[:, 1:2], scalar2=None, op0=OP.mult)
                dup = spool.tile([P, 1024], F16, tag="dup")
                tmp = spool.tile([P, 1024], F16, tag="a4b")
                for bl, sp in [(0, 0), (1, 0), (2, 64), (3, 64)]:
                    nc.sync.dma_start(out=dup[32 * bl : 32 * (bl + 1), :], in_=qpre[sp : sp + 32, :])
                nc.vector.tensor_tensor(out=tmp[:], in0=dup[:], in1=pe_a_sb[:, 1024 * hq : 1024 * (hq + 1)], op=OP.mult)
                for bl, sp in [(0, 32), (1, 32), (2, 96), (3, 96)]:
                    nc.sync.dma_start(out=dup[32 * bl : 32 * (bl + 1), :], in_=qpre[sp : sp + 32, :])
                nc.vector.tensor_tensor(out=dup[:], in0=dup[:], in1=pe_b_sb[:, 1024 * hq : 1024 * (hq + 1)], op=OP.mult)
                nc.vector.tensor_tensor(out=dst_r[:, 1024 * hq : 1024 * (hq + 1)], in0=tmp[:], in1=dup[:], op=OP.add)
            # ---- v: evict, transpose to [L, d], interleave ones ----
            vt32 = spool.tile([P, 1024], F32, tag="a8b")
            nc.vector.tensor_copy(out=vt32[:], in_=psB[:, 0:1024])
            for g in range(2):
                for i in range(4):
                    nc.tensor.matmul(
                        psB[:, 512 * g + 128 * i : 512 * g + 128 * (i + 1)],
                        lhsT=vt32[:, 128 * (4 * g + i) : 128 * (4 * g + i + 1)], rhs=ident[:],
                        is_transpose=True, start=(i == 0), stop=(i == 3),
                    )
                ch0 = 8 * hq + 4 * g
                dst = vp[:, ch0 : ch0 + 4, :].rearrange("p c (j x) -> p c j x", j=2, x=65)[:, :, :, 0:64]
                srcv = psB[:, 512 * g : 512 * (g + 1)].rearrange("p (c j x) -> p c j x", c=4, j=2, x=64)
                nc.vector.tensor_copy(out=dst, in_=srcv)

        # ============ C: attention + tail, per slab ============
        g_sb = xmt_sb  # same slot, reused as the gelu output buffer

        for s in range(NSLAB):
            for kc in range(32):
                for qh in range(2):
                    buf = (2 * kc + qh) % 2
                    base = 1024 * buf
                    qlo = 1024 * s + 512 * qh
                    nc.tensor.matmul(
                        psA[:, base : base + 512],
                        lhsT=kr[0:64, 128 * kc : 128 * (kc + 1)],
                        rhs=qr[0:64, qlo : qlo + 512],
                        start=True, stop=True, tile_position=(0, 0),
                    )
                    nc.tensor.matmul(
                        psA[:, base + 512 : base + 1024],
                        lhsT=kr[64:128, 128 * kc : 128 * (kc + 1)],
                        rhs=qr[64:128, qlo : qlo + 512],
                        start=True, stop=True, tile_position=(64, 0),
                    )
                    e_sb = spool.tile([P, 1024], BF16, tag="a4x", bufs=3)
                    nc.scalar.activation(e_sb[:], psA[:, base : base + 1024], AF.Exp, scale=0.125)
                    for h in range(2):
                        for bq in range(4):
                            b = 4 * qh + bq
                            tix = 8 * h + b
                            off = PO_OFF[tix]
                            nc.tensor.matmul(
                                psB[:, off : off + 65],
                                lhsT=e_sb[:, 512 * h + 128 * bq : 512 * h + 128 * (bq + 1)],
                                rhs=vp[:, kc, 65 * h : 65 * (h + 1)],
                                start=(kc == 0 and ((qh == 0 and tix in (0, 8)) or (qh == 1 and tix == 14))),
                                stop=(kc == 31 and qh == 1 and tix in (6, 13, 15)),
                            )
            # ---- evict + normalize + transpose + A2A ----
            po_sb = spool.tile([P, 1536], F32, tag="a8b")
            nc.vector.tensor_copy(out=po_sb[:], in_=psB[:])
            den = spool.tile([P, 16], F32, tag="den")
            nc.vector.tensor_copy(out=den[:, 0:7], in_=po_sb[:, 64:455:65])
            nc.vector.tensor_copy(out=den[:, 7:14], in_=po_sb[:, 576:967:65])
            nc.vector.tensor_copy(out=den[:, 14:16], in_=po_sb[:, 1088:1154:65])
            rec = spool.tile([P, 16], F32, tag="rec")
            nc.vector.reciprocal(rec[:], den[:])
            att = spool.tile([P, 8, P], F32, tag="a4b")
            for h in range(2):
                for b in range(8):
                    tix = 8 * h + b
                    nc.vector.tensor_scalar(
                        out=att[:, b, 64 * h : 64 * (h + 1)],
                        in0=po_sb[:, PO_OFF[tix] : PO_OFF[tix] + 64],
                        scalar1=rec[:, tix : tix + 1], scalar2=None, op0=OP.mult,
                    )
            for g in range(2):
                stage = spool.tile([P, 4, P], F16, tag="stage", bufs=2)
                for i in range(4):
                    nc.tensor.matmul(
                        psC[:, 128 * i : 128 * (i + 1)],
                        lhsT=att[:, 4 * g + i, :], rhs=ident[:],
                        is_transpose=True, start=(i == 0), stop=(i == 3),
                    )
                nc.vector.tensor_copy(out=stage[:].rearrange("p c f -> p (c f)"), in_=psC[:, 0:512])
                nc.sync.dma_start(
                    out=a2a_in[s, 4 * g : 4 * (g + 1)].rearrange("c p f -> p c f"),
                    in_=stage[:],
                )
            nc.gpsimd.collective_compute(
                "AllToAll", OP.bypass, replica_groups=RG,
                ins=[a2a_in[s].opt()], outs=[a2a_out[s].opt()],
            )

            # ---- proj + residual ----
            at_sb = spool.tile([P, KO, P], F16, tag="atsb", bufs=2)
            nc.sync.dma_start(out=at_sb[:], in_=a2a_out[s].rearrange("r p f -> p r f"))
            x2 = spool.tile([P, D], F32, tag="wqkq")
            xc2 = spool.tile([P, D], F16, tag="xrch", bufs=2)
            nc.sync.dma_start(out=xc2[:], in_=xr[s])
            for nh in range(2):
                wpq = spool.tile([P, KO, 512], F16, tag="a8", bufs=2)
                nc.sync.dma_start(
                    out=wpq[:],
                    in_=wproj[:, :, 512 * nh : 512 * (nh + 1)].rearrange("k p f -> p k f"),
                )
                for ko in range(KO):
                    nc.tensor.matmul(
                        psC[:, 0:512],
                        lhsT=at_sb[:, ko, :],
                        rhs=wpq[:, ko, :],
                        start=(ko == 0), stop=(ko == KO - 1),
                    )
                sl = slice(512 * nh, 512 * (nh + 1))
                nc.vector.tensor_tensor(out=x2[:, sl], in0=psC[:, 0:512], in1=c_pb[:, sl], op=OP.add)
                nc.vector.tensor_tensor(out=x2[:, sl], in0=x2[:, sl], in1=c_g1[:, sl], op=OP.mult)
                nc.vector.tensor_tensor(out=x2[:, sl], in0=x2[:, sl], in1=xc2[:, sl], op=OP.add)

            # ---- LN2 + modulation -> h^T ----
            h32 = spool.tile([P, D], F32, tag="wqkk")
            red2 = spool.tile([P, 6], F32, tag="red", bufs=2)
            nc.vector.reduce_sum(out=red2[:, 0:1], in_=x2[:], axis=AX.X)
            nc.vector.tensor_tensor(out=h32[:], in0=x2[:], in1=x2[:], op=OP.mult)
            nc.vector.reduce_sum(out=red2[:, 1:2], in_=h32[:], axis=AX.X)
            nc.vector.tensor_scalar(out=red2[:, 2:3], in0=red2[:, 0:1], scalar1=1.0 / D, scalar2=None, op0=OP.mult)
            nc.vector.tensor_scalar(out=red2[:, 3:4], in0=red2[:, 1:2], scalar1=1.0 / D, scalar2=None, op0=OP.mult)
            nc.vector.tensor_tensor(out=red2[:, 4:5], in0=red2[:, 2:3], in1=red2[:, 2:3], op=OP.mult)
            nc.vector.tensor_tensor(out=red2[:, 3:4], in0=red2[:, 3:4], in1=red2[:, 4:5], op=OP.subtract)
            # rstd = rsqrt(var + eps) on DVE (magic-constant + 3 Newton steps)
            nc.vector.tensor_scalar(out=red2[:, 4:5], in0=red2[:, 3:4], scalar1=EPS, scalar2=None, op0=OP.add)
            ri = spool.tile([P, 2], I32, tag="ri")
            nc.vector.tensor_scalar(out=ri[:, 0:1], in0=red2[:, 4:5].bitcast(I32), scalar1=1, scalar2=None, op0=OP.logical_shift_right)
            nc.vector.tensor_scalar(out=ri[:, 0:1], in0=ri[:, 0:1], scalar1=-1, scalar2=0x5F3759DF, op0=OP.mult, op1=OP.add)
            rf = ri[:, 0:1].bitcast(F32)
            nt = spool.tile([P, 1], F32, tag="nt")
            for _ in range(3):
                nc.vector.tensor_tensor(out=nt[:], in0=red2[:, 4:5], in1=rf, op=OP.mult)
                nc.vector.tensor_tensor(out=nt[:], in0=nt[:], in1=rf, op=OP.mult)
                nc.vector.tensor_scalar(out=nt[:], in0=nt[:], scalar1=-0.5, scalar2=1.5, op0=OP.mult, op1=OP.add)
                nc.vector.tensor_tensor(out=ri[:, 0:1].bitcast(F32), in0=rf, in1=nt[:], op=OP.mult)
            nc.vector.tensor_copy(out=red2[:, 5:6], in_=rf)
            nc.vector.tensor_scalar(
                out=h32[:], in0=x2[:], scalar1=red2[:, 2:3], scalar2=red2[:, 5:6],
                op0=OP.subtract, op1=OP.mult,
            )
            nc.vector.tensor_tensor(out=h32[:], in0=h32[:], in1=c_s2[:], op=OP.mult)
            nc.vector.tensor_tensor(out=h32[:], in0=h32[:], in1=c_sh2[:], op=OP.add)
            for g in range(2):
                for i in range(4):
                    ii = 4 * g + i
                    nc.tensor.matmul(
                        psC[:, 128 * i : 128 * (i + 1)],
                        lhsT=h32[:, 128 * ii : 128 * (ii + 1)], rhs=ident[:],
                        is_transpose=True, start=(i == 0), stop=(i == 3),
                    )
                nc.vector.tensor_copy(
                    out=hT_sb[:, 4 * g : 4 * (g + 1), :].rearrange("p c f -> p (c f)"),
                    in_=psC[:, 0:512],
                )

            # ---- MLP up (w1) + gelu ----
            gflat = g_sb[:].rearrange("p k f -> p (k f)")
            for mq in range(8):
                for mi in range(4):
                    m = 4 * mq + mi
                    for ko in range(KO):
                        nc.tensor.matmul(
                            psC[:, 128 * mi : 128 * (mi + 1)],
                            lhsT=w1_sb[:, ko, 128 * m : 128 * (m + 1)],
                            rhs=hT_sb[:, ko, :],
                            start=(mi == 0 and ko == 0),
                            stop=(mi == 3 and ko == KO - 1),
                        )
                ptmp = spool.tile([P, 512], F32, tag="ptmp", bufs=2)
                nc.vector.tensor_copy(out=ptmp[:], in_=psC[:, 0:512])
                xg4 = spool.tile([P, 512], F16, tag="xg4")
                for mi in range(4):
                    m = 4 * mq + mi
                    nc.vector.tensor_scalar(
                        out=xg4[:, 128 * mi : 128 * (mi + 1)],
                        in0=ptmp[:, 128 * mi : 128 * (mi + 1)],
                        scalar1=b1_sb[:, m : m + 1], scalar2=None, op0=OP.add,
                    )
                u4 = spool.tile([P, 512], F16, tag="u4")
                nc.vector.tensor_tensor(out=u4[:], in0=xg4[:], in1=xg4[:], op=OP.mult)
                nc.vector.tensor_scalar(out=u4[:], in0=u4[:], scalar1=G1 / 2, scalar2=G0 / 2, op0=OP.mult, op1=OP.add)
                nc.vector.tensor_tensor(out=u4[:], in0=xg4[:], in1=u4[:], op=OP.mult)
                d4 = spool.tile([P, 512], F32, tag="d4", bufs=2)
                nc.scalar.activation(d4[:], u4[:], AF.Tanh)
                nc.vector.tensor_scalar(out=d4[:], in0=d4[:], scalar1=0.5, scalar2=0.5, op0=OP.mult, op1=OP.add)
                nc.vector.tensor_tensor(
                    out=gflat[:, 512 * mq : 512 * (mq + 1)],
                    in0=xg4[:], in1=d4[:], op=OP.mult,
                )

            # ---- MLP down (w2) + residual -> y ----
            out_sb = spool.tile([P, D], F32, tag="outsb")
            for nh in range(2):
                for mq4 in range(4):
                    w2q = spool.tile([P, 8, 512], F16, tag="a8", bufs=2)
                    nc.sync.dma_start(
                        out=w2q[:],
                        in_=wm2[8 * mq4 : 8 * (mq4 + 1), :, 512 * nh : 512 * (nh + 1)].rearrange("m p f -> p m f"),
                    )
                    for mm in range(8):
                        m = 8 * mq4 + mm
                        nc.tensor.matmul(
                            psC[:, 0:512],
                            lhsT=gflat[:, 128 * m : 128 * (m + 1)],
                            rhs=w2q[:, mm, :],
                            start=(m == 0), stop=(m == 31),
                        )
                sl = slice(512 * nh, 512 * (nh + 1))
                nc.vector.tensor_tensor(out=out_sb[:, sl], in0=psC[:, 0:512], in1=c_b2[:, sl], op=OP.add)
                nc.vector.tensor_tensor(out=out_sb[:, sl], in0=out_sb[:, sl], in1=c_g2[:, sl], op=OP.mult)
                nc.vector.tensor_tensor(out=out_sb[:, sl], in0=out_sb[:, sl], in1=x2[:, sl], op=OP.add)
            nc.sync.dma_start(out=y[s], in_=out_sb[:])

    nc.compile()
    return nc


# ======================= host side =======================

_PROG = None


def _get_program():
    global _PROG
    if _PROG is None:
        _PROG = build_program()
    return _PROG


def _qk_cols(h0):
    idx = []
    for hh in (h0, h0 + 1):
        idx += [hh * 64 + 2 * p for p in range(32)]
        idx += [hh * 64 + 2 * p + 1 for p in range(32)]
    return idx


def _prep_core(c, x_full, vec, pe0, w):
    f16 = np.float16
    d = {}
    chunks = [g_chunk(c, s) for s in range(NSLAB)]
    d["xr"] = np.stack([x_full[128 * g : 128 * (g + 1)] for g in chunks]).astype(f16)
    d["sv"] = vec.reshape(KO, P).T.astype(np.float32).copy()
    st = "cond" if c < 2 else "obs"
    d["wmod"] = w[f"{st}_mod_w"].reshape(KO, P, 6 * D).astype(f16)
    d["bmod"] = w[f"{st}_mod_b"].reshape(1, 6 * D).astype(np.float32)
    qc = _qk_cols(2 * c)
    kc = [D + j for j in qc]
    vc = [2 * D + 64 * (2 * c) + i for i in range(128)]
    wqk = np.stack(
        [
            np.concatenate([w["cond_qkv_w"][:, qc], w["cond_qkv_w"][:, kc]], axis=1),
            np.concatenate([w["obs_qkv_w"][:, qc], w["obs_qkv_w"][:, kc]], axis=1),
        ]
    )
    d["wqk"] = wqk.reshape(2, KO, P, 256).astype(f16)
    wvv = np.stack([w["cond_qkv_w"][:, vc], w["obs_qkv_w"][:, vc]])
    d["wv"] = wvv.reshape(2, KO, P, P).astype(f16)
    perm = np.concatenate([np.arange(128 * g, 128 * (g + 1)) for g in PI])
    peP = pe0[perm]
    pair = np.arange(P) % 32
    jout = (np.arange(P) // 32) % 2
    d["pe_a"] = peP[:, pair, jout, 0].T.astype(f16).copy()
    d["pe_b"] = peP[:, pair, jout, 1].T.astype(f16).copy()
    dmap = 2 * (np.arange(P) % 32) + ((np.arange(P) // 32) % 2)
    d["qsc"] = np.stack([w["cond_q_scale"][dmap], w["obs_q_scale"][dmap]], axis=1).astype(np.float32).copy()
    d["ksc"] = np.stack([w["cond_k_scale"][dmap], w["obs_k_scale"][dmap]], axis=1).astype(np.float32).copy()
    d["wproj"] = w[f"{st}_proj_w"].reshape(KO, P, D).astype(f16)
    d["wm1"] = w[f"{st}_mlp_w1"].reshape(KO, P, MH).astype(f16)
    d["wm2"] = w[f"{st}_mlp_w2"].reshape(NM, P, D).astype(f16)
    d["b1c"] = w[f"{st}_mlp_b1"].reshape(NM, P).T.astype(np.float32).copy()
    d["pb2"] = np.concatenate([w[f"{st}_proj_b"], w[f"{st}_mlp_b2"]]).reshape(1, 2 * D).astype(np.float32)
    return d


def kernel(**inputs):
    nc = _get_program()
    from concourse.bass_utils import run_bass_kernel_spmd

    w = {k: np.asarray(v) for k, v in inputs.items()}
    obs = w["obs"][0].astype(np.float32)
    cond = w["cond"][0].astype(np.float32)
    x_full = np.concatenate([cond, obs], axis=0)
    vec = w["vec"][0].astype(np.float32)
    pe0 = w["pe"][0, 0].astype(np.float32)

    in_maps = [_prep_core(c, x_full, vec, pe0, w) for c in range(NC)]
    res = run_bass_kernel_spmd(nc, in_maps, list(range(NC)), trace=False)

    out_full = np.zeros((L, D), np.float32)
    for r in range(NC):
        yr = res.results[r]["y"]
        for s in range(NSLAB):
            g = g_chunk(r, s)
            out_full[128 * g : 128 * (g + 1)] = yr[s]
    return out_full[1024:][None], out_full[:1024][None]


# revision 20
# speedup vs baseline: 1.1731x; 1.0870x over previous
"""DoubleStreamBlock (flux-style) kernel for 8 trn2 NeuronCores.

Sharding plan:
  - The concatenated (cond, obs) sequence (4096 rows x 1024) is split into
    32 chunks of 128 rows; core r owns 4 stream-pure chunks g(r, s), s=0..3.
  - modulation + LN1 run row-sharded; the modulated activations are
    transposed and AllGathered (fp16) so every core holds full xm^T.
  - qkv / rmsnorm / rope / attention are head-sharded (2 of 16 heads per
    core, full L=4096).  Queries are processed in 4 slabs of 1024 in a
    permuted order such that each slab contains exactly one 128-row chunk
    per core; after each slab a small AllToAll returns attention rows to
    their owner so proj + LN2 + MLP of slab s overlap the (ScalarE-bound)
    softmax exp of slab s+1.
  - proj and MLP run row-sharded with full fp16 weights per core; no
    AllReduce anywhere.
Matmuls are fp16 (bf16 softmax numerator) with fp32 accumulation; all
norm / softmax arithmetic is fp32.
"""

import sys

if "/opt/trn_rl_repo" not in sys.path:
    sys.path.insert(0, "/opt/trn_rl_repo")

import numpy as np

import concourse.bass as bass
import concourse.mybir as mybir
import concourse.tile as tile
from concourse import bacc
from concourse.masks import make_identity

F16 = mybir.dt.float16
I32 = mybir.dt.int32
BF16 = mybir.dt.bfloat16
F32 = mybir.dt.float32
AF = mybir.ActivationFunctionType
OP = mybir.AluOpType
AX = mybir.AxisListType

P = 128
D = 1024
H = 16
DH = 64
MH = 4096
L = 4096
NC = 8
NSLAB = 4
KO = D // P
NM = MH // P
EPS = 1e-6
G0 = float(2.0 * np.sqrt(2.0 / np.pi))
G1 = float(2.0 * np.sqrt(2.0 / np.pi) * 0.044715)


def g_chunk(r, s):
    return 2 * s + r if r < 2 else 8 + 6 * s + (r - 2)


PI = [g_chunk(r, s) for s in range(NSLAB) for r in range(NC)]



_ACT_PATCHED = False


def _patch_act_tables():
    global _ACT_PATCHED
    if _ACT_PATCHED:
        return
    import concourse.bacc as _bacc_mod
    from concourse import hw_specs as _hw

    _orig = _hw.get_activation_tables

    def _patched(arch):
        t = _orig(arch)
        AFT = mybir.ActivationFunctionType
        for name, fns in t.items():
            if name not in ("natural_log_exp_and_others", "exp_and_others"):
                fns.discard(AFT.Exp)
            if name != "natural_log_exp_and_others":
                fns.discard(AFT.Ln)
            if name != "exp_and_others":
                fns.discard(AFT.Tanh)
        return t

    _bacc_mod.get_activation_tables = _patched
    _ACT_PATCHED = True


def build_program():
    _patch_act_tables()
    nc = bacc.Bacc("TRN2", target_bir_lowering=False, debug=False, num_devices=NC)

    # ---------------- I/O ----------------
    xr = nc.dram_tensor("xr", [NSLAB, P, D], F16, kind="ExternalInput")
    sv = nc.dram_tensor("sv", [P, KO], F32, kind="ExternalInput")
    wmod = nc.dram_tensor("wmod", [KO, P, 6 * D], F16, kind="ExternalInput")
    bmod = nc.dram_tensor("bmod", [1, 6 * D], F32, kind="ExternalInput")
    wqk = nc.dram_tensor("wqk", [2, KO, P, 256], F16, kind="ExternalInput")
    wv = nc.dram_tensor("wv", [2, KO, P, P], F16, kind="ExternalInput")
    pe_a = nc.dram_tensor("pe_a", [P, L], F16, kind="ExternalInput")
    pe_b = nc.dram_tensor("pe_b", [P, L], F16, kind="ExternalInput")
    qsc = nc.dram_tensor("qsc", [P, 2], F32, kind="ExternalInput")
    ksc = nc.dram_tensor("ksc", [P, 2], F32, kind="ExternalInput")
    wproj = nc.dram_tensor("wproj", [KO, P, D], F16, kind="ExternalInput")
    wm1 = nc.dram_tensor("wm1", [KO, P, MH], F16, kind="ExternalInput")
    wm2 = nc.dram_tensor("wm2", [NM, P, D], F16, kind="ExternalInput")
    b1c = nc.dram_tensor("b1c", [P, NM], F32, kind="ExternalInput")
    pb2 = nc.dram_tensor("pb2", [1, 2 * D], F32, kind="ExternalInput")
    y = nc.dram_tensor("y", [NSLAB, P, D], F32, kind="ExternalOutput")

    # internal DRAM
    modd = nc.dram_tensor("modd", [1, 6 * D], F32)
    agx_in = nc.dram_tensor("agx_in", [KO, P, 512], F16)
    agx_out = nc.dram_tensor("agx_out", [NC, KO, P, 512], F16, addr_space="Shared")
    a2a_in = nc.dram_tensor("a2a_in", [NSLAB, NC, P, P], F16)
    a2a_out = nc.dram_tensor("a2a_out", [NSLAB, NC, P, P], F16)

    # PSUM: 8 banks
    psA = nc.alloc_psum_tensor("psA", [P, 2048], F32)   # banks 0-3
    psB = nc.alloc_psum_tensor("psB", [P, 1536], F32)   # banks 4-6
    psC = nc.alloc_psum_tensor("psC", [P, 512], F32)    # bank 7

    RG = [list(range(NC))]
    PO_OFF = [65 * t if t < 7 else 512 + 65 * (t - 7) if t < 14 else 1024 + 65 * (t - 14)
              for t in range(16)]

    with tile.TileContext(nc) as tc, \
         tc.tile_pool(name="consts", bufs=1) as cpool, \
         tc.tile_pool(name="big", bufs=1) as bpool, \
         tc.tile_pool(name="scr", bufs=1) as spool:

        # ----- small persistent consts -----
        ident = cpool.tile([P, P], F32, tag="ident")
        make_identity(nc, ident[:])
        qsc_sb = cpool.tile([P, 2], F32, tag="qsc")
        ksc_sb = cpool.tile([P, 2], F32, tag="ksc")
        b1_sb = cpool.tile([P, NM], F32, tag="b1")
        ln1a = cpool.tile([P, KO], F32, tag="ln1a")
        ln1b = cpool.tile([P, KO], F32, tag="ln1b")
        eps_t = cpool.tile([P, 1], F32, tag="epst")
        nc.vector.memset(eps_t[:], EPS)
        c_g1 = cpool.tile([P, D], F16, tag="cg1")
        c_pb = cpool.tile([P, D], F16, tag="cpb")
        c_s2 = cpool.tile([P, D], F16, tag="cs2")
        c_sh2 = cpool.tile([P, D], F16, tag="csh2")
        c_g2 = cpool.tile([P, D], F16, tag="cg2")
        c_b2 = cpool.tile([P, D], F16, tag="cb2")
        bd1 = cpool.tile([P, P], F16, tag="bd1")
        nc.vector.memset(bd1[:], 0.0)
        nc.vector.memset(bd1[0:64, 0:64], 1.0)
        nc.vector.memset(bd1[64:128, 64:128], 1.0)

        # ----- big persistent -----
        qr = bpool.tile([P, L], F16, tag="qr")
        kr = bpool.tile([P, L], F16, tag="kr")
        vp = bpool.tile([P, 32, 130], BF16, tag="vp")
        w1_sb = bpool.tile([P, KO, MH], F16, tag="w1")
        hT_sb = bpool.tile([P, KO, P], F16, tag="hT")

        # ============ A0: silu(vec) + modulation matvec ============
        sv_sb = cpool.tile([P, KO], F32, tag="sv")
        nc.sync.dma_start(out=sv_sb[:], in_=sv[:])
        sil = cpool.tile([P, KO], F32, tag="sil")
        nc.scalar.activation(sil[:], sv_sb[:], AF.Exp, scale=-1.0)
        nc.vector.tensor_scalar(out=sil[:], in0=sil[:], scalar1=1.0, scalar2=None, op0=OP.add)
        silr = cpool.tile([P, KO], F32, tag="silr")
        nc.vector.reciprocal(silr[:], sil[:])
        s_sb = cpool.tile([P, KO], F16, tag="ssb")
        nc.vector.tensor_tensor(out=s_sb[:], in0=sv_sb[:], in1=silr[:], op=OP.mult)

        for rnd in range(3):
            for ko in range(KO):
                wmt = spool.tile([P, 2048], F16, tag="a4x", bufs=3)
                nc.sync.dma_start(out=wmt[:], in_=wmod[ko, :, 2048 * rnd : 2048 * (rnd + 1)])
                for g in range(4):
                    nc.tensor.matmul(
                        psA[0:1, 512 * g : 512 * (g + 1)],
                        lhsT=s_sb[:, ko : ko + 1],
                        rhs=wmt[:, 512 * g : 512 * (g + 1)],
                        start=(ko == 0), stop=(ko == KO - 1),
                    )
            bmt = spool.tile([1, 2048], F32, tag="a8b")
            nc.sync.dma_start(out=bmt[:], in_=bmod[0:1, 2048 * rnd : 2048 * (rnd + 1)])
            nc.vector.tensor_tensor(out=bmt[:], in0=psA[0:1, 0:2048], in1=bmt[:], op=OP.add)
            nc.sync.dma_start(out=modd[0:1, 2048 * rnd : 2048 * (rnd + 1)], in_=bmt[:])

        # ============ A1: broadcast const tiles from mod vectors ============
        for dst, lo in [(c_g1, 2 * D), (c_sh2, 3 * D), (c_s2, 4 * D), (c_g2, 5 * D)]:
            nc.gpsimd.dma_start(out=dst[:], in_=modd[0, lo : lo + D][None, :].broadcast_to((P, D)))
        nc.vector.tensor_scalar(out=c_s2[:], in0=c_s2[:], scalar1=1.0, scalar2=None, op0=OP.add)
        nc.sync.dma_start(out=ln1b[:], in_=modd[0, 0:D].rearrange("(k p) -> p k", p=P))
        nc.sync.dma_start(out=ln1a[:], in_=modd[0, D : 2 * D].rearrange("(k p) -> p k", p=P))
        nc.vector.tensor_scalar(out=ln1a[:], in0=ln1a[:], scalar1=1.0, scalar2=None, op0=OP.add)

        # ============ A2: LN1 -> xm^T -> AllGather ============
        xmt_sb = bpool.tile([P, KO, 512], F16, tag="gxmt")  # reused as g_sb later
        for s in range(NSLAB):
            xch = spool.tile([P, D], F16, tag="xrch", bufs=2)
            nc.sync.dma_start(out=xch[:], in_=xr[s])
            t1 = spool.tile([P, D], F32, tag="a4x", bufs=3)
            sqv = spool.tile([P, D], F32, tag="a4b")
            red = spool.tile([P, 6], F32, tag="red", bufs=2)
            nc.vector.reduce_sum(out=red[:, 0:1], in_=xch[:], axis=AX.X)
            nc.vector.tensor_tensor(out=sqv[:], in0=xch[:], in1=xch[:], op=OP.mult)
            nc.vector.reduce_sum(out=red[:, 1:2], in_=sqv[:], axis=AX.X)
            nc.vector.tensor_scalar(out=red[:, 2:3], in0=red[:, 0:1], scalar1=1.0 / D, scalar2=None, op0=OP.mult)
            nc.vector.tensor_scalar(out=red[:, 3:4], in0=red[:, 1:2], scalar1=1.0 / D, scalar2=None, op0=OP.mult)
            nc.vector.tensor_tensor(out=red[:, 4:5], in0=red[:, 2:3], in1=red[:, 2:3], op=OP.mult)
            nc.vector.tensor_tensor(out=red[:, 3:4], in0=red[:, 3:4], in1=red[:, 4:5], op=OP.subtract)
            nc.scalar.activation(red[:, 4:5], red[:, 3:4], AF.Ln, bias=eps_t[:])
            nc.scalar.activation(red[:, 5:6], red[:, 4:5], AF.Exp, scale=-0.5)
            nc.vector.tensor_scalar(
                out=t1[:], in0=xch[:], scalar1=red[:, 2:3], scalar2=red[:, 5:6],
                op0=OP.subtract, op1=OP.mult,
            )
            for g in range(2):
                for i in range(4):
                    ii = 4 * g + i
                    nc.tensor.matmul(
                        psA[:, 512 * g + 128 * i : 512 * g + 128 * (i + 1)],
                        lhsT=t1[:, 128 * ii : 128 * (ii + 1)], rhs=ident[:],
                        is_transpose=True, start=(i == 0), stop=(i == 3),
                    )
                for i in range(4):
                    ii = 4 * g + i
                    nc.vector.tensor_scalar(
                        out=xmt_sb[:, ii, 128 * s : 128 * (s + 1)],
                        in0=psA[:, 512 * g + 128 * i : 512 * g + 128 * (i + 1)],
                        scalar1=ln1a[:, ii : ii + 1], scalar2=ln1b[:, ii : ii + 1],
                        op0=OP.mult, op1=OP.add,
                    )
        nc.sync.dma_start(out=agx_in[:].rearrange("k p f -> p k f"), in_=xmt_sb[:])
        nc.gpsimd.collective_compute(
            "AllGather", OP.bypass, replica_groups=RG,
            ins=[agx_in[:].opt()], outs=[agx_out[:].opt()],
        )

        # qkv weights + pe planes (issued early so DMA overlaps phase A tail)
        pe_a_sb = spool.tile([P, L], F16, tag="a8", bufs=2)
        pe_b_sb = spool.tile([P, L], F16, tag="a8", bufs=2)
        wqkq = spool.tile([P, 2, KO, P], F16, tag="wqkq")
        wqkk = spool.tile([P, 2, KO, P], F16, tag="wqkk")
        wv_sb = spool.tile([P, 2, KO, P], F16, tag="wvsb")
        nc.sync.dma_start(out=wqkq[:], in_=wqk[:, :, :, 0:128].rearrange("t k p f -> p t k f"))
        nc.sync.dma_start(out=wqkk[:], in_=wqk[:, :, :, 128:256].rearrange("t k p f -> p t k f"))
        nc.sync.dma_start(out=wv_sb[:], in_=wv[:].rearrange("t k p f -> p t k f"))
        nc.sync.dma_start(out=pe_a_sb[:], in_=pe_a[:])
        nc.sync.dma_start(out=pe_b_sb[:], in_=pe_b[:])
        nc.sync.dma_start(out=qsc_sb[:], in_=qsc[:])
        nc.sync.dma_start(out=ksc_sb[:], in_=ksc[:])
        nc.sync.dma_start(out=b1_sb[:], in_=b1c[:])
        nc.gpsimd.dma_start(out=c_pb[:], in_=pb2[0, 0:D][None, :].broadcast_to((P, D)))
        nc.gpsimd.dma_start(out=c_b2[:], in_=pb2[0, D : 2 * D][None, :].broadcast_to((P, D)))
        for ko in range(KO):
            nc.sync.dma_start(out=w1_sb[:, ko, :], in_=wm1[ko])

        # ============ B: qkv (head-sharded) ============
        CGRP = [(0, 0), (256, 1), (512, 1), (768, 1)]

        nc.vector.memset(vp[:, :, 64:65], 1.0)
        nc.vector.memset(vp[:, :, 129:130], 1.0)
        for hq in range(4):          # one pi slab (1024 cols) per pass
            for ko in range(KO):
                xt = spool.tile([P, 1024], F16, tag="a4x", bufs=3)
                nc.sync.dma_start(
                    out=xt[:].rearrange("p (r i) -> p r i", r=NC),
                    in_=agx_out[:, ko, :, 128 * hq : 128 * (hq + 1)].rearrange("r p i -> p r i"),
                )
                for t, wt in [(0, wqkq), (1, wqkk), (2, wv_sb)]:
                    for (off, st) in CGRP:
                        if t == 0:
                            dst = psA[:, off : off + 256]
                        elif t == 1:
                            dst = psA[:, 1024 + off : 1024 + off + 256]
                        else:
                            dst = psB[:, off : off + 256]
                        nc.tensor.matmul(
                            dst,
                            lhsT=wt[:, st, ko, :],
                            rhs=xt[:, off : off + 256],
                            start=(ko == 0 and off % 512 == 0),
                            stop=(ko == KO - 1 and (off + 256) % 512 == 0),
                        )
            # ---- q then k: evict, rmsnorm, rope ----
            for t, (dst_r, scv) in enumerate([(qr, qsc_sb), (kr, ksc_sb)]):
                ev = spool.tile([P, 1024], F32, tag="a8b")
                nc.vector.tensor_copy(out=ev[:], in_=psA[:, 1024 * t : 1024 * (t + 1)])
                sq16 = spool.tile([P, 1024], F16, tag="a4c")
                nc.vector.tensor_tensor(out=sq16[:], in0=ev[:], in1=ev[:], op=OP.mult)
                nc.tensor.matmul(psB[:, 1024:1536], lhsT=bd1[:], rhs=sq16[:, 0:512], start=True, stop=True)
                nc.tensor.matmul(psC[:, 0:512], lhsT=bd1[:], rhs=sq16[:, 512:1024], start=True, stop=True)
                rst = spool.tile([P, 1024], F16, tag="a4b")
                nc.scalar.activation(rst[:, 0:512], psB[:, 1024:1536], AF.Ln, scale=1.0 / DH, bias=eps_t[:])
                nc.scalar.activation(rst[:, 512:1024], psC[:, 0:512], AF.Ln, scale=1.0 / DH, bias=eps_t[:])
                nc.scalar.activation(rst[:], rst[:], AF.Exp, scale=-0.5)
                qpre = spool.tile([P, 1024], F16, tag="a4c")
                nc.vector.tensor_tensor(out=qpre[:], in0=ev[:], in1=rst[:], op=OP.mult)
                nc.vector.tensor_scalar(out=qpre[:, 0:256], in0=qpre[:, 0:256],
                                        scalar1=scv[:, 0:1], scalar2=None, op0=OP.mult)
                nc.vector.tensor_scalar(out=qpre[:, 256:1024], in0=qpre[:, 256:1024],
                                        scalar1=scv[:, 1:2], scalar2=None, op0=OP.mult)
                dup = spool.tile([P, 1024], F16, tag="dup")
                tmp = spool.tile([P, 1024], F16, tag="a4b")
                for bl, sp in [(0, 0), (1, 0), (2, 64), (3, 64)]:
                    nc.sync.dma_start(out=dup[32 * bl : 32 * (bl + 1), :], in_=qpre[sp : sp + 32, :])
                nc.vector.tensor_tensor(out=tmp[:], in0=dup[:], in1=pe_a_sb[:, 1024 * hq : 1024 * (hq + 1)], op=OP.mult)
                for bl, sp in [(0, 32), (1, 32), (2, 96), (3, 96)]:
                    nc.sync.dma_start(out=dup[32 * bl : 32 * (bl + 1), :], in_=qpre[sp : sp + 32, :])
                nc.vector.tensor_tensor(out=dup[:], in0=dup[:], in1=pe_b_sb[:, 1024 * hq : 1024 * (hq + 1)], op=OP.mult)
                nc.vector.tensor_tensor(out=dst_r[:, 1024 * hq : 1024 * (hq + 1)], in0=tmp[:], in1=dup[:], op=OP.add)
            # ---- v: evict, transpose to [L, d], interleave ones ----
            vt32 = spool.tile([P, 1024], F32, tag="a8b")
            nc.vector.tensor_copy(out=vt32[:], in_=psB[:, 0:1024])
            for g in range(2):
                for i in range(4):
                    nc.tensor.matmul(
                        psB[:, 512 * g + 128 * i : 512 * g + 128 * (i + 1)],
                        lhsT=vt32[:, 128 * (4 * g + i) : 128 * (4 * g + i + 1)], rhs=ident[:],
                        is_transpose=True, start=(i == 0), stop=(i == 3),
                    )
                ch0 = 8 * hq + 4 * g
                dst = vp[:, ch0 : ch0 + 4, :].rearrange("p c (j x) -> p c j x", j=2, x=65)[:, :, :, 0:64]
                srcv = psB[:, 512 * g : 512 * (g + 1)].rearrange("p (c j x) -> p c j x", c=4, j=2, x=64)
                nc.vector.tensor_copy(out=dst, in_=srcv)

        # ============ C: attention + tail (interleaved), per slab ============
        g_sb = xmt_sb  # same slot, reused as the gelu output buffer
        gflat = g_sb[:].rearrange("p k f -> p (k f)")

        def attn_step(s, kc, qh):
            buf = (2 * kc + qh) % 2
            base = 1024 * buf
            qlo = 1024 * s + 512 * qh
            nc.tensor.matmul(
                psA[:, base : base + 512],
                lhsT=kr[0:64, 128 * kc : 128 * (kc + 1)],
                rhs=qr[0:64, qlo : qlo + 512],
                start=True, stop=True, tile_position=(0, 0),
            )
            nc.tensor.matmul(
                psA[:, base + 512 : base + 1024],
                lhsT=kr[64:128, 128 * kc : 128 * (kc + 1)],
                rhs=qr[64:128, qlo : qlo + 512],
                start=True, stop=True, tile_position=(64, 0),
            )
            e_sb = spool.tile([P, 1024], BF16, tag="a4x", bufs=3)
            nc.scalar.activation(e_sb[:], psA[:, base : base + 1024], AF.Exp, scale=0.125)
            for h in range(2):
                for bq in range(4):
                    b = 4 * qh + bq
                    tix = 8 * h + b
                    off = PO_OFF[tix]
                    nc.tensor.matmul(
                        psB[:, off : off + 65],
                        lhsT=e_sb[:, 512 * h + 128 * bq : 512 * h + 128 * (bq + 1)],
                        rhs=vp[:, kc, 65 * h : 65 * (h + 1)],
                        start=(kc == 0 and ((qh == 0 and tix in (0, 8)) or (qh == 1 and tix == 14))),
                        stop=(kc == 31 and qh == 1 and tix in (6, 13, 15)),
                    )

        def slab_finish(s):
            po_sb = spool.tile([P, 1536], F32, tag="a8b")
            nc.vector.tensor_copy(out=po_sb[:], in_=psB[:])
            den = spool.tile([P, 16], F32, tag="den")
            nc.vector.tensor_copy(out=den[:, 0:7], in_=po_sb[:, 64:455:65])
            nc.vector.tensor_copy(out=den[:, 7:14], in_=po_sb[:, 576:967:65])
            nc.vector.tensor_copy(out=den[:, 14:16], in_=po_sb[:, 1088:1154:65])
            rec = spool.tile([P, 16], F32, tag="rec")
            nc.vector.reciprocal(rec[:], den[:])
            att = spool.tile([P, 8, P], F32, tag="a4b")
            for h in range(2):
                for b in range(8):
                    tix = 8 * h + b
                    nc.vector.tensor_scalar(
                        out=att[:, b, 64 * h : 64 * (h + 1)],
                        in0=po_sb[:, PO_OFF[tix] : PO_OFF[tix] + 64],
                        scalar1=rec[:, tix : tix + 1], scalar2=None, op0=OP.mult,
                    )
            for g in range(2):
                stage = spool.tile([P, 4, P], F16, tag="stage", bufs=2)
                for i in range(4):
                    nc.tensor.matmul(
                        psC[:, 128 * i : 128 * (i + 1)],
                        lhsT=att[:, 4 * g + i, :], rhs=ident[:],
                        is_transpose=True, start=(i == 0), stop=(i == 3),
                    )
                nc.vector.tensor_copy(out=stage[:].rearrange("p c f -> p (c f)"), in_=psC[:, 0:512])
                nc.sync.dma_start(
                    out=a2a_in[s, 4 * g : 4 * (g + 1)].rearrange("c p f -> p c f"),
                    in_=stage[:],
                )
            nc.gpsimd.collective_compute(
                "AllToAll", OP.bypass, replica_groups=RG,
                ins=[a2a_in[s].opt()], outs=[a2a_out[s].opt()],
            )

        def tail_gen(s):
            # proj + residual
            at_sb = spool.tile([P, KO, P], F16, tag="atsb", bufs=2)
            nc.sync.dma_start(out=at_sb[:], in_=a2a_out[s].rearrange("r p f -> p r f"))
            x2 = spool.tile([P, D], F32, tag="wqkq")
            xc2 = spool.tile([P, D], F16, tag="xrch", bufs=2)
            nc.sync.dma_start(out=xc2[:], in_=xr[s])
            wpq0 = spool.tile([P, KO, 512], F16, tag="a8", bufs=2)
            nc.sync.dma_start(out=wpq0[:], in_=wproj[:, :, 0:512].rearrange("k p f -> p k f"))
            yield
            wpq1 = spool.tile([P, KO, 512], F16, tag="a8", bufs=2)
            nc.sync.dma_start(out=wpq1[:], in_=wproj[:, :, 512:1024].rearrange("k p f -> p k f"))
            yield
            for nh, wpq in ((0, wpq0), (1, wpq1)):
                for ko in range(KO):
                    nc.tensor.matmul(
                        psC[:, 0:512],
                        lhsT=at_sb[:, ko, :],
                        rhs=wpq[:, ko, :],
                        start=(ko == 0), stop=(ko == KO - 1),
                    )
                sl = slice(512 * nh, 512 * (nh + 1))
                nc.vector.tensor_tensor(out=x2[:, sl], in0=psC[:, 0:512], in1=c_pb[:, sl], op=OP.add)
                nc.vector.tensor_tensor(out=x2[:, sl], in0=x2[:, sl], in1=c_g1[:, sl], op=OP.mult)
                nc.vector.tensor_tensor(out=x2[:, sl], in0=x2[:, sl], in1=xc2[:, sl], op=OP.add)
                yield
            # LN2 + modulation
            h32 = spool.tile([P, D], F32, tag="wqkk")
            red2 = spool.tile([P, 6], F32, tag="red", bufs=2)
            nc.vector.reduce_sum(out=red2[:, 0:1], in_=x2[:], axis=AX.X)
            nc.vector.tensor_tensor(out=h32[:], in0=x2[:], in1=x2[:], op=OP.mult)
            nc.vector.reduce_sum(out=red2[:, 1:2], in_=h32[:], axis=AX.X)
            nc.vector.tensor_scalar(out=red2[:, 2:3], in0=red2[:, 0:1], scalar1=1.0 / D, scalar2=None, op0=OP.mult)
            nc.vector.tensor_scalar(out=red2[:, 3:4], in0=red2[:, 1:2], scalar1=1.0 / D, scalar2=None, op0=OP.mult)
            nc.vector.tensor_tensor(out=red2[:, 4:5], in0=red2[:, 2:3], in1=red2[:, 2:3], op=OP.mult)
            nc.vector.tensor_tensor(out=red2[:, 3:4], in0=red2[:, 3:4], in1=red2[:, 4:5], op=OP.subtract)
            yield
            # rstd = rsqrt(var + eps) on DVE (magic + 3 Newton steps)
            nc.vector.tensor_scalar(out=red2[:, 4:5], in0=red2[:, 3:4], scalar1=EPS, scalar2=None, op0=OP.add)
            ri = spool.tile([P, 2], I32, tag="ri")
            nc.vector.tensor_scalar(out=ri[:, 0:1], in0=red2[:, 4:5].bitcast(I32), scalar1=1, scalar2=None, op0=OP.logical_shift_right)
            nc.vector.tensor_scalar(out=ri[:, 0:1], in0=ri[:, 0:1], scalar1=-1, scalar2=0x5F3759DF, op0=OP.mult, op1=OP.add)
            rf = ri[:, 0:1].bitcast(F32)
            nt = spool.tile([P, 1], F32, tag="nt")
            for _ in range(3):
                nc.vector.tensor_tensor(out=nt[:], in0=red2[:, 4:5], in1=rf, op=OP.mult)
                nc.vector.tensor_tensor(out=nt[:], in0=nt[:], in1=rf, op=OP.mult)
                nc.vector.tensor_scalar(out=nt[:], in0=nt[:], scalar1=-0.5, scalar2=1.5, op0=OP.mult, op1=OP.add)
                nc.vector.tensor_tensor(out=ri[:, 0:1].bitcast(F32), in0=rf, in1=nt[:], op=OP.mult)
            nc.vector.tensor_copy(out=red2[:, 5:6], in_=rf)
            nc.vector.tensor_scalar(
                out=h32[:], in0=x2[:], scalar1=red2[:, 2:3], scalar2=red2[:, 5:6],
                op0=OP.subtract, op1=OP.mult,
            )
            nc.vector.tensor_tensor(out=h32[:], in0=h32[:], in1=c_s2[:], op=OP.mult)
            nc.vector.tensor_tensor(out=h32[:], in0=h32[:], in1=c_sh2[:], op=OP.add)
            yield
            for g in range(2):
                for i in range(4):
                    ii = 4 * g + i
                    nc.tensor.matmul(
                        psC[:, 128 * i : 128 * (i + 1)],
                        lhsT=h32[:, 128 * ii : 128 * (ii + 1)], rhs=ident[:],
                        is_transpose=True, start=(i == 0), stop=(i == 3),
                    )
                nc.vector.tensor_copy(
                    out=hT_sb[:, 4 * g : 4 * (g + 1), :].rearrange("p c f -> p (c f)"),
                    in_=psC[:, 0:512],
                )
                yield
            # MLP up (w1) + gelu
            for mq in range(8):
                for mi in range(4):
                    m = 4 * mq + mi
                    for ko in range(KO):
                        nc.tensor.matmul(
                            psC[:, 128 * mi : 128 * (mi + 1)],
                            lhsT=w1_sb[:, ko, 128 * m : 128 * (m + 1)],
                            rhs=hT_sb[:, ko, :],
                            start=(mi == 0 and ko == 0),
                            stop=(mi == 3 and ko == KO - 1),
                        )
                ptmp = spool.tile([P, 512], F32, tag="ptmp", bufs=2)
                nc.vector.tensor_copy(out=ptmp[:], in_=psC[:, 0:512])
                xg4 = spool.tile([P, 512], F16, tag="xg4")
                for mi in range(4):
                    m = 4 * mq + mi
                    nc.vector.tensor_scalar(
                        out=xg4[:, 128 * mi : 128 * (mi + 1)],
                        in0=ptmp[:, 128 * mi : 128 * (mi + 1)],
                        scalar1=b1_sb[:, m : m + 1], scalar2=None, op0=OP.add,
                    )
                u4 = spool.tile([P, 512], F16, tag="u4")
                nc.vector.tensor_tensor(out=u4[:], in0=xg4[:], in1=xg4[:], op=OP.mult)
                nc.vector.tensor_scalar(out=u4[:], in0=u4[:], scalar1=G1 / 2, scalar2=G0 / 2, op0=OP.mult, op1=OP.add)
                nc.vector.tensor_tensor(out=u4[:], in0=xg4[:], in1=u4[:], op=OP.mult)
                d4 = spool.tile([P, 512], F32, tag="d4", bufs=2)
                nc.scalar.activation(d4[:], u4[:], AF.Tanh)
                nc.vector.tensor_scalar(out=d4[:], in0=d4[:], scalar1=0.5, scalar2=0.5, op0=OP.mult, op1=OP.add)
                nc.vector.tensor_tensor(
                    out=gflat[:, 512 * mq : 512 * (mq + 1)],
                    in0=xg4[:], in1=d4[:], op=OP.mult,
                )
                yield
            # MLP down (w2) + residual -> y
            out_sb = spool.tile([P, D], F32, tag="outsb")
            w2qs = {}
            w2qs[(0, 0)] = spool.tile([P, 8, 512], F16, tag="a8", bufs=2, name="w2q00")
            nc.sync.dma_start(
                out=w2qs[(0, 0)][:],
                in_=wm2[0:8, :, 0:512].rearrange("m p f -> p m f"),
            )
            yield
            quads = [(nh, mq4) for nh in range(2) for mq4 in range(4)]
            for qi, (nh, mq4) in enumerate(quads):
                if qi + 1 < len(quads):
                    nxt = quads[qi + 1]
                    w2qs[nxt] = spool.tile([P, 8, 512], F16, tag="a8", bufs=2, name=f"w2q{qi + 1}")
                    nc.sync.dma_start(
                        out=w2qs[nxt][:],
                        in_=wm2[8 * nxt[1] : 8 * nxt[1] + 8, :, 512 * nxt[0] : 512 * nxt[0] + 512].rearrange("m p f -> p m f"),
                    )
                w2q = w2qs.pop((nh, mq4))
                for mm in range(8):
                    m = 8 * mq4 + mm
                    nc.tensor.matmul(
                        psC[:, 0:512],
                        lhsT=gflat[:, 128 * m : 128 * (m + 1)],
                        rhs=w2q[:, mm, :],
                        start=(m == 0), stop=(m == 31),
                    )
                if mq4 == 3:
                    sl = slice(512 * nh, 512 * (nh + 1))
                    nc.vector.tensor_tensor(out=out_sb[:, sl], in0=psC[:, 0:512], in1=c_b2[:, sl], op=OP.add)
                    nc.vector.tensor_tensor(out=out_sb[:, sl], in0=out_sb[:, sl], in1=c_g2[:, sl], op=OP.mult)
                    nc.vector.tensor_tensor(out=out_sb[:, sl], in0=out_sb[:, sl], in1=x2[:, sl], op=OP.add)
                yield
            nc.sync.dma_start(out=y[s], in_=out_sb[:])
            yield

        pending = None
        for s in range(NSLAB):
            cnt = 0
            for kc in range(32):
                for qh in range(2):
                    attn_step(s, kc, qh)
                    cnt += 1
                    if pending is not None and cnt % 2 == 0:
                        try:
                            next(pending)
                        except StopIteration:
                            pending = None
            if pending is not None:
                for _ in pending:
                    pass
                pending = None
            slab_finish(s)
            pending = tail_gen(s)
        for _ in pending:
            pass

    nc.compile()
    return nc


# ======================= host side =======================

_PROG = None


def _get_program():
    global _PROG
    if _PROG is None:
        _PROG = build_program()
    return _PROG


def _qk_cols(h0):
    idx = []
    for hh in (h0, h0 + 1):
        idx += [hh * 64 + 2 * p for p in range(32)]
        idx += [hh * 64 + 2 * p + 1 for p in range(32)]
    return idx


def _prep_core(c, x_full, vec, pe0, w):
    f16 = np.float16
    d = {}
    chunks = [g_chunk(c, s) for s in range(NSLAB)]
    d["xr"] = np.stack([x_full[128 * g : 128 * (g + 1)] for g in chunks]).astype(f16)
    d["sv"] = vec.reshape(KO, P).T.astype(np.float32).copy()
    st = "cond" if c < 2 else "obs"
    d["wmod"] = w[f"{st}_mod_w"].reshape(KO, P, 6 * D).astype(f16)
    d["bmod"] = w[f"{st}_mod_b"].reshape(1, 6 * D).astype(np.float32)
    qc = _qk_cols(2 * c)
    kc = [D + j for j in qc]
    vc = [2 * D + 64 * (2 * c) + i for i in range(128)]
    wqk = np.stack(
        [
            np.concatenate([w["cond_qkv_w"][:, qc], w["cond_qkv_w"][:, kc]], axis=1),
            np.concatenate([w["obs_qkv_w"][:, qc], w["obs_qkv_w"][:, kc]], axis=1),
        ]
    )
    d["wqk"] = wqk.reshape(2, KO, P, 256).astype(f16)
    wvv = np.stack([w["cond_qkv_w"][:, vc], w["obs_qkv_w"][:, vc]])
    d["wv"] = wvv.reshape(2, KO, P, P).astype(f16)
    perm = np.concatenate([np.arange(128 * g, 128 * (g + 1)) for g in PI])
    peP = pe0[perm]
    pair = np.arange(P) % 32
    jout = (np.arange(P) // 32) % 2
    d["pe_a"] = peP[:, pair, jout, 0].T.astype(f16).copy()
    d["pe_b"] = peP[:, pair, jout, 1].T.astype(f16).copy()
    dmap = 2 * (np.arange(P) % 32) + ((np.arange(P) // 32) % 2)
    d["qsc"] = np.stack([w["cond_q_scale"][dmap], w["obs_q_scale"][dmap]], axis=1).astype(np.float32).copy()
    d["ksc"] = np.stack([w["cond_k_scale"][dmap], w["obs_k_scale"][dmap]], axis=1).astype(np.float32).copy()
    d["wproj"] = w[f"{st}_proj_w"].reshape(KO, P, D).astype(f16)
    d["wm1"] = w[f"{st}_mlp_w1"].reshape(KO, P, MH).astype(f16)
    d["wm2"] = w[f"{st}_mlp_w2"].reshape(NM, P, D).astype(f16)
    d["b1c"] = w[f"{st}_mlp_b1"].reshape(NM, P).T.astype(np.float32).copy()
    d["pb2"] = np.concatenate([w[f"{st}_proj_b"], w[f"{st}_mlp_b2"]]).reshape(1, 2 * D).astype(np.float32)
    return d


def kernel(**inputs):
    nc = _get_program()
    from concourse.bass_utils import run_bass_kernel_spmd

    w = {k: np.asarray(v) for k, v in inputs.items()}
    obs = w["obs"][0].astype(np.float32)
    cond = w["cond"][0].astype(np.float32)
    x_full = np.concatenate([cond, obs], axis=0)
    vec = w["vec"][0].astype(np.float32)
    pe0 = w["pe"][0, 0].astype(np.float32)

    in_maps = [_prep_core(c, x_full, vec, pe0, w) for c in range(NC)]
    res = run_bass_kernel_spmd(nc, in_maps, list(range(NC)), trace=False)

    out_full = np.zeros((L, D), np.float32)
    for r in range(NC):
        yr = res.results[r]["y"]
        for s in range(NSLAB):
            g = g_chunk(r, s)
            out_full[128 * g : 128 * (g + 1)] = yr[s]
    return out_full[1024:][None], out_full[:1024][None]
